# revision 11
# baseline (speedup 1.0000x reference)
"""BRGCN (2-layer relational GAT) for Trainium2, 8 NeuronCores.

Strategy (graph/data parallel per sharding hint): layer-0 targets are
sharded contiguously across the 8 cores. The FLOP-dominant dense block --
the per-relation Q/K/V projections of the aggregated messages z
([R=5, 15000, 256] @ [5, 256, 256] x3, ~30 GF) -- runs on the device,
each core owning 1875 target nodes. The irregular, index-dependent
message passing (edge gather, per-(target,relation) softmax,
scatter-add) is prepared around it.

The device kernel is DMA-bound (memory regime): all device I/O is bf16
(fp32 PSUM accumulation), halving HBM traffic vs fp32 -- ~21 MB/core.
Stationary weights are reused across column chunks, PSUM is evacuated
with explicit VectorE copies (casting to bf16), and the four column
chunks of each output row-block are staged into one SBUF tile so each
output DMA is a single large transfer.

Only the first 30000 rows of x and the first 15000 rows of x1 can affect
the output (edge indices are bounded by N1/N2), so everything else is
skipped.
"""
import os
import sys
import numpy as np

for _p in ("/opt/trn_rl_repo", "/root/.axon_site/_ro/trn_rl_repo"):
    if os.path.isdir(_p) and _p not in sys.path:
        sys.path.insert(0, _p)

import ml_dtypes
import concourse.bass as bass
import concourse.bacc as bacc
import concourse.mybir as mybir
import concourse.tile as tile
from concourse.bass_utils import run_bass_kernel_spmd

R = 5
NEG_SLOPE = 0.2
N1 = 30000
N2 = 15000
NCORES = 8
NPC = N2 // NCORES          # 1875 target nodes per core
NPAD = 1880                 # padded to 4 chunks of 470
NCHUNK = 470
HC0 = 256
BF16 = ml_dtypes.bfloat16

LAST_RESULTS = None         # BassKernelResults of the device launch

_compiled = None


def _ensure_ntff_hook():
    """bass_utils' trace path imports antenv.axon_hooks, which this image's
    antenv package lacks. Inject an equivalent in-memory module wired to
    the axon PJRT .so so NTFF profiling (exec_time_ns) works. Returns True
    if the trace path is usable."""
    try:
        import antenv.axon_hooks  # noqa: F401
        return True
    except ImportError:
        pass
    try:
        import types
        import antenv
        from trn_agent_boot.trn_boot import _ntff_profile_via_ctypes
        hook = _ntff_profile_via_ctypes("/opt/axon/libaxon_pjrt.so")
        mod = types.ModuleType("antenv.axon_hooks")
        state = {"hook": hook}
        mod.get_axon_ntff_profile_hook = lambda: state["hook"]
        mod.set_axon_ntff_profile_hook = lambda h: state.update(hook=h)
        sys.modules["antenv.axon_hooks"] = mod
        antenv.axon_hooks = mod
        return hook is not None
    except Exception as e:
        sys.stderr.write(f"[kernel] ntff hook setup failed ({e!r})\n")
        return False


def _build_device_program():
    """Per-core program: for r in 0..4, q/k/v^T = W^T @ z_r^T, all bf16 I/O.

    Inputs  zT  [5, 256, 1880] bf16   (this core's z, transposed, padded)
            wq/wk/wv [5, 256, 256] bf16
    Outputs qT/kT/vT [5, 256, 1880] bf16
    """
    nc = bacc.Bacc("TRN2", target_bir_lowering=False, debug=False,
                   num_devices=NCORES)
    bf16 = mybir.dt.bfloat16
    f32 = mybir.dt.float32

    zT = nc.declare_dram_parameter("zT", [R, HC0, NPAD], bf16, isOutput=False)
    ws = [nc.declare_dram_parameter(n, [R, HC0, HC0], bf16, isOutput=False)
          for n in ("wq", "wk", "wv")]
    outs = [nc.declare_dram_parameter(n, [R, HC0, NPAD], bf16, isOutput=True)
            for n in ("qT", "kT", "vT")]

    with tile.TileContext(nc) as tc:
        with (
            tc.tile_pool(name="zp", bufs=5) as zp,
            tc.tile_pool(name="wp", bufs=1) as wp,
            tc.tile_pool(name="op", bufs=4) as op,
            tc.tile_pool(name="tp", bufs=4) as tp,
            tc.tile_pool(name="ps", bufs=8, space="PSUM") as psp,
        ):
            # Preload every weight tile once (no slot reuse anywhere on the
            # input side): each HW-queue DMA then carries at most the FIFO
            # wait -- this walrus build rejects DMAs with >1 sem wait.
            wts = {}
            for wi, w in enumerate(ws):
                for r in range(R):
                    for kc in range(2):
                        t = wp.tile([128, HC0], bf16, tag=f"w{wi}_{r}_{kc}",
                                    name="w")
                        nc.sync.dma_start(out=t[:], in_=w[r, kc * 128:(kc + 1) * 128, :])
                        wts[(wi, r, kc)] = t
            for r in range(R):
                zt = []
                for kc in range(2):
                    t = zp.tile([128, NPAD], bf16, tag=f"z{kc}")
                    nc.sync.dma_start(out=t[:], in_=zT[r, kc * 128:(kc + 1) * 128, :])
                    zt.append(t)
                for wi, o in enumerate(outs):
                    for mc in range(2):
                        ps = [psp.tile([128, NCHUNK], f32, tag="acc", name="acc")
                              for _ in range(4)]
                        # kc outer / nch inner: the stationary weight tile is
                        # reused across the 4 column chunks
                        for kc in range(2):
                            for nch in range(4):
                                nc.tensor.matmul(
                                    out=ps[nch][:],
                                    lhsT=wts[(wi, r, kc)][:, mc * 128:(mc + 1) * 128],
                                    rhs=zt[kc][:, nch * NCHUNK:(nch + 1) * NCHUNK],
                                    start=(kc == 0),
                                    stop=(kc == 1),
                                )
                        ot = op.tile([128, NPAD], bf16, tag="out")
                        for nch in range(4):
                            nc.vector.tensor_copy(
                                out=ot[:, nch * NCHUNK:(nch + 1) * NCHUNK],
                                in_=ps[nch][:],
                            )
                        # ACT touch op overlapping all 4 DVE writes: absorbs
                        # the cross-engine dependency on an engine
                        # instruction (multi-wait OK), so the store DMA
                        # issued next from ACT needs no producer wait.
                        touch = tp.tile([128, 4], bf16, tag="touch")
                        nc.scalar.copy(out=touch[:],
                                       in_=ot[:, 0:NPAD:NCHUNK])
                        nc.scalar.dma_start(
                            out=o[r, mc * 128:(mc + 1) * 128, :],
                            in_=ot[:],
                        )
    nc.finalize()   # Bacc.compile(): legalizes multi-sem waits (1/inst on TRN2)
    return nc


def _device_qkv(z):
    """z [R, N2, 256] f32 -> q, k, v [R, N2, 256] via the 8-core kernel."""
    global _compiled, LAST_RESULTS
    if _compiled is None:
        _compiled = _build_device_program()
    nc = _compiled
    zb = z.astype(BF16)
    in_maps = []
    for d in range(NCORES):
        zs = zb[:, d * NPC:(d + 1) * NPC, :]                 # [5, 1875, 256]
        zt = np.zeros((R, HC0, NPAD), dtype=BF16)
        zt[:, :, :NPC] = zs.transpose(0, 2, 1)
        in_maps.append({"zT": zt, "wq": _W[0], "wk": _W[1], "wv": _W[2]})
    res = run_bass_kernel_spmd(
        nc, in_maps, list(range(NCORES)),
        trace=bool(os.environ.get("KERNEL_TRACE")) and _ensure_ntff_hook(),
    )
    LAST_RESULTS = res
    q = np.empty((R, N2, HC0), dtype=np.float32)
    k = np.empty((R, N2, HC0), dtype=np.float32)
    v = np.empty((R, N2, HC0), dtype=np.float32)
    for d in range(NCORES):
        rd = res.results[d]
        sl = slice(d * NPC, (d + 1) * NPC)
        q[:, sl, :] = rd["qT"][:, :, :NPC].transpose(0, 2, 1).astype(np.float32)
        k[:, sl, :] = rd["kT"][:, :, :NPC].transpose(0, 2, 1).astype(np.float32)
        v[:, sl, :] = rd["vT"][:, :, :NPC].transpose(0, 2, 1).astype(np.float32)
    return q, k, v


_W = None


def _seg_softmax_scatter(alpha, xj, seg, nseg, hc):
    """Edge softmax grouped by seg, then weighted scatter-add of xj.

    Sort-by-segment + reduceat: identical math to segment_max/segment_sum
    (empty segments yield zero rows), much faster than np.add.at.
    """
    E, H = alpha.shape
    order = np.argsort(seg, kind="stable")
    seg_s = seg[order]
    alpha_s = alpha[order]
    starts = np.flatnonzero(np.r_[True, seg_s[1:] != seg_s[:-1]])
    uniq = seg_s[starts]
    amax = np.zeros((nseg, H), dtype=np.float32)
    amax[uniq] = np.maximum.reduceat(alpha_s, starts, axis=0)
    ex_s = np.exp(alpha_s - amax[seg_s], dtype=np.float32)
    den = np.zeros((nseg, H), dtype=np.float32)
    den[uniq] = np.add.reduceat(ex_s, starts, axis=0)
    w_s = ex_s / np.maximum(den[seg_s], 1e-16)
    msg_s = (w_s[:, :, None] * xj[order].reshape(E, H, -1)).reshape(E, hc)
    z = np.zeros((nseg, hc), dtype=np.float32)
    z[uniq] = np.add.reduceat(msg_s.astype(np.float32), starts, axis=0)
    return z


def _relation_attention(z, q, k, v, Wrel, heads, outc, N):
    hc = heads * outc
    qh = q.reshape(R, N, heads, outc)
    kh = k.reshape(R, N, heads, outc)
    vh = v.reshape(R, N, heads, outc)
    psi = np.einsum("rnhc,snhc->rsnh", qh, kh).astype(np.float32)
    mask = (psi == 0) & (np.sum(psi, axis=1, keepdims=True) != 0)
    psi_m = np.where(mask, -np.inf, psi)
    pm = np.max(psi_m, axis=1, keepdims=True)
    pe = np.exp(psi_m - pm, dtype=np.float32)
    prob = pe / np.sum(pe, axis=1, keepdims=True)
    delta = np.einsum("rsnh,snhc->rnhc", prob, vh).reshape(R, N, hc)
    return np.einsum("rnd,r->nd", delta, Wrel[:, 0]).astype(np.float32)


def kernel(**inputs):
    global _W
    I = {k: np.asarray(val) for k, val in inputs.items()}
    emb = I["emb"].astype(np.float32)
    nid = I["n_id"].astype(np.int64)
    lni = I["local_node_idx"].astype(np.int64)

    # ---- group_input (only the 30000 rows that matter)
    x = emb[lni[nid[:N1]]]                                   # [30000, 128]

    # ---- layer 0: per-relation GAT over edges with tgt < 15000
    ei0 = I["edge_index0"].astype(np.int64)
    et0 = I["edge_type0"].astype(np.int64)
    keep = ei0[1] < N2
    src, tgt, rel = ei0[0][keep], ei0[1][keep], et0[keep]

    Wj0, Wi0 = I["Wj0"].astype(np.float32), I["Wi0"].astype(np.float32)
    att_j0, att_i0 = I["att_j0"].astype(np.float32), I["att_i0"].astype(np.float32)
    hj = (x @ Wj0).astype(np.float32)                        # [30000, 256]
    hi = (x[:N2] @ Wi0).astype(np.float32)                   # [15000, 256]
    H0, C0 = 4, 64
    xj = hj[src]                                             # [E, 256]
    xi = hi[tgt]
    aj = np.einsum("ehc,ehc->eh", att_j0[rel], xj.reshape(-1, H0, C0))
    ai = np.einsum("ehc,ehc->eh", att_i0[rel], xi.reshape(-1, H0, C0))
    s = (aj + ai).astype(np.float32)
    alpha = np.where(s >= 0, s, NEG_SLOPE * s).astype(np.float32)
    seg = tgt * R + rel
    z = _seg_softmax_scatter(alpha, xj, seg, N2 * R, HC0)
    z = z.reshape(N2, R, HC0).transpose(1, 0, 2)             # [5, 15000, 256]

    # ---- device: per-relation Q/K/V projections (the dominant dense block)
    _W = (np.ascontiguousarray(I["Wq0"].astype(np.float32)).astype(BF16),
          np.ascontiguousarray(I["Wk0"].astype(np.float32)).astype(BF16),
          np.ascontiguousarray(I["Wv0"].astype(np.float32)).astype(BF16))
    try:
        q, k, v = _device_qkv(z)
    except Exception as e:  # device unavailable -> host fallback, stays correct
        sys.stderr.write(f"[kernel] device path failed ({e!r}); host fallback\n")
        W = [w.astype(np.float32) for w in _W]
        q = np.einsum("rnd,rde->rne", z, W[0]).astype(np.float32)
        k = np.einsum("rnd,rde->rne", z, W[1]).astype(np.float32)
        v = np.einsum("rnd,rde->rne", z, W[2]).astype(np.float32)

    out0 = _relation_attention(z, q, k, v, I["Wrel0"].astype(np.float32), H0, C0, N2)
    x1 = out0 + x[:N2] @ I["sw0"].astype(np.float32) + I["sb0"].astype(np.float32)
    x1 = np.maximum(x1, 0.0).astype(np.float32)              # [15000, 256]

    # ---- layer 1 (small: 40-dim), host
    ei1 = I["edge_index1"].astype(np.int64)
    et1 = I["edge_type1"].astype(np.int64)
    src1, tgt1, rel1 = ei1[0], ei1[1], et1
    Wj1, Wi1 = I["Wj1"].astype(np.float32), I["Wi1"].astype(np.float32)
    hj1 = (x1 @ Wj1).astype(np.float32)                      # [15000, 40]
    hi1 = (x1[:N2] @ Wi1).astype(np.float32)
    H1, C1 = 1, 40
    xj1 = hj1[src1]
    xi1 = hi1[tgt1]
    aj1 = np.einsum("ehc,ehc->eh", I["att_j1"].astype(np.float32)[rel1],
                    xj1.reshape(-1, H1, C1))
    ai1 = np.einsum("ehc,ehc->eh", I["att_i1"].astype(np.float32)[rel1],
                    xi1.reshape(-1, H1, C1))
    s1 = (aj1 + ai1).astype(np.float32)
    alpha1 = np.where(s1 >= 0, s1, NEG_SLOPE * s1).astype(np.float32)
    seg1 = tgt1 * R + rel1
    z1 = _seg_softmax_scatter(alpha1, xj1, seg1, N2 * R, C1)
    z1 = z1.reshape(N2, R, C1).transpose(1, 0, 2)            # [5, 15000, 40]

    q1 = np.einsum("rnd,rde->rne", z1, I["Wq1"].astype(np.float32))
    k1 = np.einsum("rnd,rde->rne", z1, I["Wk1"].astype(np.float32))
    v1 = np.einsum("rnd,rde->rne", z1, I["Wv1"].astype(np.float32))
    out1 = _relation_attention(z1, q1, k1, v1, I["Wrel1"].astype(np.float32),
                               H1, C1, N2)
    x2 = out1 + x1 @ I["sw1"].astype(np.float32) + I["sb1"].astype(np.float32)

    # ---- log_softmax
    m = np.max(x2, axis=-1, keepdims=True)
    e = np.exp(x2 - m, dtype=np.float32)
    return (x2 - m - np.log(np.sum(e, axis=-1, keepdims=True))).astype(np.float32)


# revision 12
# speedup vs baseline: 1.1312x; 1.1312x over previous
"""BRGCN (2-layer relational GAT) for Trainium2, 8 NeuronCores.

Strategy (graph/data parallel per sharding hint): layer-0 targets are
sharded contiguously across the 8 cores. The FLOP-dominant dense block --
the per-relation Q/K/V projections of the aggregated messages z
([R=5, 15000, 256] @ [5, 256, 256] x3, ~30 GF) -- runs on the device,
each core owning 1875 target nodes. The irregular, index-dependent
message passing (edge gather, per-(target,relation) softmax,
scatter-add) is prepared around it.

The device kernel is DMA-bound (memory regime): all device I/O is bf16
(fp32 PSUM accumulation), halving HBM traffic vs fp32 -- ~21 MB/core.
Stationary weights are reused across column chunks, PSUM is evacuated
with explicit VectorE copies (casting to bf16), and the four column
chunks of each output row-block are staged into one SBUF tile so each
output DMA is a single large transfer.

Only the first 30000 rows of x and the first 15000 rows of x1 can affect
the output (edge indices are bounded by N1/N2), so everything else is
skipped.
"""
import os
import sys
import numpy as np

for _p in ("/opt/trn_rl_repo", "/root/.axon_site/_ro/trn_rl_repo"):
    if os.path.isdir(_p) and _p not in sys.path:
        sys.path.insert(0, _p)

import ml_dtypes
import concourse.bass as bass
import concourse.bacc as bacc
import concourse.mybir as mybir
import concourse.tile as tile
from concourse.bass_utils import run_bass_kernel_spmd

R = 5
NEG_SLOPE = 0.2
N1 = 30000
N2 = 15000
NCORES = 8
NPC = N2 // NCORES          # 1875 target nodes per core
NPAD = 1880                 # padded to 4 chunks of 470
NCHUNK = 470
HC0 = 256
BF16 = ml_dtypes.bfloat16

LAST_RESULTS = None         # BassKernelResults of the device launch

_compiled = None


def _ensure_ntff_hook():
    """bass_utils' trace path imports antenv.axon_hooks, which this image's
    antenv package lacks. Inject an equivalent in-memory module wired to
    the axon PJRT .so so NTFF profiling (exec_time_ns) works. Returns True
    if the trace path is usable."""
    try:
        import antenv.axon_hooks  # noqa: F401
        return True
    except ImportError:
        pass
    try:
        import types
        import antenv
        from trn_agent_boot.trn_boot import _ntff_profile_via_ctypes
        hook = _ntff_profile_via_ctypes("/opt/axon/libaxon_pjrt.so")
        mod = types.ModuleType("antenv.axon_hooks")
        state = {"hook": hook}
        mod.get_axon_ntff_profile_hook = lambda: state["hook"]
        mod.set_axon_ntff_profile_hook = lambda h: state.update(hook=h)
        sys.modules["antenv.axon_hooks"] = mod
        antenv.axon_hooks = mod
        return hook is not None
    except Exception as e:
        sys.stderr.write(f"[kernel] ntff hook setup failed ({e!r})\n")
        return False


def _build_device_program():
    """Per-core program: for r in 0..4, q/k/v^T = W^T @ z_r^T, all bf16 I/O.

    Inputs  zT  [5, 256, 1880] bf16   (this core's z, transposed, padded)
            wq/wk/wv [5, 256, 256] bf16
    Outputs qT/kT/vT [5, 256, 1880] bf16
    """
    nc = bacc.Bacc("TRN2", target_bir_lowering=False, debug=False,
                   num_devices=NCORES)
    bf16 = mybir.dt.bfloat16
    f32 = mybir.dt.float32

    zT = nc.declare_dram_parameter("zT", [R, HC0, NPAD], bf16, isOutput=False)
    ws = [nc.declare_dram_parameter(n, [R, HC0, HC0], bf16, isOutput=False)
          for n in ("wq", "wk", "wv")]
    outs = [nc.declare_dram_parameter(n, [R, HC0, NPAD], bf16, isOutput=True)
            for n in ("qT", "kT", "vT")]

    with tile.TileContext(nc) as tc:
        with (
            tc.tile_pool(name="zp", bufs=1) as zp,
            tc.tile_pool(name="wp", bufs=1) as wp,
            tc.tile_pool(name="sp", bufs=4) as sp,
            tc.tile_pool(name="tp", bufs=4) as tp,
            tc.tile_pool(name="ps", bufs=8, space="PSUM") as psp,
        ):
            # Few LARGE input DMAs (HW-queue DMAs are completion-serialized
            # by the FIFO sem scheme, ~0.6-1us each): one per (w-tensor,
            # k-chunk) and one per z k-chunk, batched across all relations.
            # Ordered so the first matmul's operands land first.
            wts = {}
            zt = {}

            def load_w(wi, kc):
                t = wp.tile([128, R * HC0], bf16, tag=f"w{wi}_{kc}", name="w")
                nc.sync.dma_start(
                    out=t[:].rearrange("p (r m) -> p r m", r=R),
                    in_=ws[wi][:, kc * 128:(kc + 1) * 128, :]
                        .rearrange("r p m -> p r m"),
                )
                wts[(wi, kc)] = t

            def load_z(kc):
                t = zp.tile([128, R * NPAD], bf16, tag=f"z{kc}", name="z")
                nc.sync.dma_start(
                    out=t[:].rearrange("p (r n) -> p r n", r=R),
                    in_=zT[:, kc * 128:(kc + 1) * 128, :]
                        .rearrange("r p n -> p r n"),
                )
                zt[kc] = t

            load_w(0, 0)
            load_z(0)
            load_w(0, 1)
            load_z(1)
            for wi in (1, 2):
                for kc in range(2):
                    load_w(wi, kc)

            for wi, o in enumerate(outs):
                for r in range(R):
                    # stage both mc row-blocks of (wi, r), then store with
                    # ONE DMA; stores ride the ACT ring and overlap compute
                    st = sp.tile([128, 2 * NPAD], bf16, tag="stage",
                                 name="st")
                    for mc in range(2):
                        ps = [psp.tile([128, NCHUNK], f32, tag="acc",
                                       name="acc") for _ in range(4)]
                        # kc outer / nch inner: stationary weight reused
                        # across the 4 column chunks
                        for kc in range(2):
                            for nch in range(4):
                                nc.tensor.matmul(
                                    out=ps[nch][:],
                                    lhsT=wts[(wi, kc)][:, r * HC0 + mc * 128:
                                                       r * HC0 + mc * 128 + 128],
                                    rhs=zt[kc][:, r * NPAD + nch * NCHUNK:
                                               r * NPAD + (nch + 1) * NCHUNK],
                                    start=(kc == 0),
                                    stop=(kc == 1),
                                )
                        # PSUM evacuation split 3:1 DVE:ACT (DVE is the
                        # serial floor; ACT is otherwise idle)
                        for nch in range(4):
                            dst = st[:, mc * NPAD + nch * NCHUNK:
                                     mc * NPAD + (nch + 1) * NCHUNK]
                            if nch == 3:
                                nc.scalar.copy(out=dst, in_=ps[nch][:])
                            else:
                                nc.vector.tensor_copy(out=dst, in_=ps[nch][:])
                    # ACT touch op strided across every chunk of the staging
                    # tile: absorbs the DVE dependency on an engine
                    # instruction, so the store DMA issued next from ACT in
                    # program order needs no producer wait (this walrus
                    # accepts at most 1 sem wait per instruction; Bacc
                    # legalizes the rest into EventSemaphores).
                    touch = tp.tile([128, 8], bf16, tag="touch", name="tch")
                    nc.scalar.copy(out=touch[:],
                                   in_=st[:, 0:2 * NPAD:NCHUNK])
                    nc.scalar.dma_start(
                        out=o[r].rearrange("(mc p) n -> p mc n", mc=2),
                        in_=st[:].rearrange("p (mc n) -> p mc n", mc=2),
                    )
    nc.finalize()   # Bacc.compile(): legalizes multi-sem waits (1/inst on TRN2)
    return nc


def _device_qkv(z):
    """z [R, N2, 256] f32 -> q, k, v [R, N2, 256] via the 8-core kernel."""
    global _compiled, LAST_RESULTS
    if _compiled is None:
        _compiled = _build_device_program()
    nc = _compiled
    zb = z.astype(BF16)
    in_maps = []
    for d in range(NCORES):
        zs = zb[:, d * NPC:(d + 1) * NPC, :]                 # [5, 1875, 256]
        zt = np.zeros((R, HC0, NPAD), dtype=BF16)
        zt[:, :, :NPC] = zs.transpose(0, 2, 1)
        in_maps.append({"zT": zt, "wq": _W[0], "wk": _W[1], "wv": _W[2]})
    res = run_bass_kernel_spmd(
        nc, in_maps, list(range(NCORES)),
        trace=bool(os.environ.get("KERNEL_TRACE")) and _ensure_ntff_hook(),
    )
    LAST_RESULTS = res
    q = np.empty((R, N2, HC0), dtype=np.float32)
    k = np.empty((R, N2, HC0), dtype=np.float32)
    v = np.empty((R, N2, HC0), dtype=np.float32)
    for d in range(NCORES):
        rd = res.results[d]
        sl = slice(d * NPC, (d + 1) * NPC)
        q[:, sl, :] = rd["qT"][:, :, :NPC].transpose(0, 2, 1).astype(np.float32)
        k[:, sl, :] = rd["kT"][:, :, :NPC].transpose(0, 2, 1).astype(np.float32)
        v[:, sl, :] = rd["vT"][:, :, :NPC].transpose(0, 2, 1).astype(np.float32)
    return q, k, v


_W = None


def _seg_softmax_scatter(alpha, xj, seg, nseg, hc):
    """Edge softmax grouped by seg, then weighted scatter-add of xj.

    Sort-by-segment + reduceat: identical math to segment_max/segment_sum
    (empty segments yield zero rows), much faster than np.add.at.
    """
    E, H = alpha.shape
    order = np.argsort(seg, kind="stable")
    seg_s = seg[order]
    alpha_s = alpha[order]
    starts = np.flatnonzero(np.r_[True, seg_s[1:] != seg_s[:-1]])
    uniq = seg_s[starts]
    amax = np.zeros((nseg, H), dtype=np.float32)
    amax[uniq] = np.maximum.reduceat(alpha_s, starts, axis=0)
    ex_s = np.exp(alpha_s - amax[seg_s], dtype=np.float32)
    den = np.zeros((nseg, H), dtype=np.float32)
    den[uniq] = np.add.reduceat(ex_s, starts, axis=0)
    w_s = ex_s / np.maximum(den[seg_s], 1e-16)
    msg_s = (w_s[:, :, None] * xj[order].reshape(E, H, -1)).reshape(E, hc)
    z = np.zeros((nseg, hc), dtype=np.float32)
    z[uniq] = np.add.reduceat(msg_s.astype(np.float32), starts, axis=0)
    return z


def _relation_attention(z, q, k, v, Wrel, heads, outc, N):
    hc = heads * outc
    qh = q.reshape(R, N, heads, outc)
    kh = k.reshape(R, N, heads, outc)
    vh = v.reshape(R, N, heads, outc)
    psi = np.einsum("rnhc,snhc->rsnh", qh, kh).astype(np.float32)
    mask = (psi == 0) & (np.sum(psi, axis=1, keepdims=True) != 0)
    psi_m = np.where(mask, -np.inf, psi)
    pm = np.max(psi_m, axis=1, keepdims=True)
    pe = np.exp(psi_m - pm, dtype=np.float32)
    prob = pe / np.sum(pe, axis=1, keepdims=True)
    delta = np.einsum("rsnh,snhc->rnhc", prob, vh).reshape(R, N, hc)
    return np.einsum("rnd,r->nd", delta, Wrel[:, 0]).astype(np.float32)


def kernel(**inputs):
    global _W
    I = {k: np.asarray(val) for k, val in inputs.items()}
    emb = I["emb"].astype(np.float32)
    nid = I["n_id"].astype(np.int64)
    lni = I["local_node_idx"].astype(np.int64)

    # ---- group_input (only the 30000 rows that matter)
    x = emb[lni[nid[:N1]]]                                   # [30000, 128]

    # ---- layer 0: per-relation GAT over edges with tgt < 15000
    ei0 = I["edge_index0"].astype(np.int64)
    et0 = I["edge_type0"].astype(np.int64)
    keep = ei0[1] < N2
    src, tgt, rel = ei0[0][keep], ei0[1][keep], et0[keep]

    Wj0, Wi0 = I["Wj0"].astype(np.float32), I["Wi0"].astype(np.float32)
    att_j0, att_i0 = I["att_j0"].astype(np.float32), I["att_i0"].astype(np.float32)
    hj = (x @ Wj0).astype(np.float32)                        # [30000, 256]
    hi = (x[:N2] @ Wi0).astype(np.float32)                   # [15000, 256]
    H0, C0 = 4, 64
    xj = hj[src]                                             # [E, 256]
    xi = hi[tgt]
    aj = np.einsum("ehc,ehc->eh", att_j0[rel], xj.reshape(-1, H0, C0))
    ai = np.einsum("ehc,ehc->eh", att_i0[rel], xi.reshape(-1, H0, C0))
    s = (aj + ai).astype(np.float32)
    alpha = np.where(s >= 0, s, NEG_SLOPE * s).astype(np.float32)
    seg = tgt * R + rel
    z = _seg_softmax_scatter(alpha, xj, seg, N2 * R, HC0)
    z = z.reshape(N2, R, HC0).transpose(1, 0, 2)             # [5, 15000, 256]

    # ---- device: per-relation Q/K/V projections (the dominant dense block)
    _W = (np.ascontiguousarray(I["Wq0"].astype(np.float32)).astype(BF16),
          np.ascontiguousarray(I["Wk0"].astype(np.float32)).astype(BF16),
          np.ascontiguousarray(I["Wv0"].astype(np.float32)).astype(BF16))
    try:
        q, k, v = _device_qkv(z)
    except Exception as e:  # device unavailable -> host fallback, stays correct
        sys.stderr.write(f"[kernel] device path failed ({e!r}); host fallback\n")
        W = [w.astype(np.float32) for w in _W]
        q = np.einsum("rnd,rde->rne", z, W[0]).astype(np.float32)
        k = np.einsum("rnd,rde->rne", z, W[1]).astype(np.float32)
        v = np.einsum("rnd,rde->rne", z, W[2]).astype(np.float32)

    out0 = _relation_attention(z, q, k, v, I["Wrel0"].astype(np.float32), H0, C0, N2)
    x1 = out0 + x[:N2] @ I["sw0"].astype(np.float32) + I["sb0"].astype(np.float32)
    x1 = np.maximum(x1, 0.0).astype(np.float32)              # [15000, 256]

    # ---- layer 1 (small: 40-dim), host
    ei1 = I["edge_index1"].astype(np.int64)
    et1 = I["edge_type1"].astype(np.int64)
    src1, tgt1, rel1 = ei1[0], ei1[1], et1
    Wj1, Wi1 = I["Wj1"].astype(np.float32), I["Wi1"].astype(np.float32)
    hj1 = (x1 @ Wj1).astype(np.float32)                      # [15000, 40]
    hi1 = (x1[:N2] @ Wi1).astype(np.float32)
    H1, C1 = 1, 40
    xj1 = hj1[src1]
    xi1 = hi1[tgt1]
    aj1 = np.einsum("ehc,ehc->eh", I["att_j1"].astype(np.float32)[rel1],
                    xj1.reshape(-1, H1, C1))
    ai1 = np.einsum("ehc,ehc->eh", I["att_i1"].astype(np.float32)[rel1],
                    xi1.reshape(-1, H1, C1))
    s1 = (aj1 + ai1).astype(np.float32)
    alpha1 = np.where(s1 >= 0, s1, NEG_SLOPE * s1).astype(np.float32)
    seg1 = tgt1 * R + rel1
    z1 = _seg_softmax_scatter(alpha1, xj1, seg1, N2 * R, C1)
    z1 = z1.reshape(N2, R, C1).transpose(1, 0, 2)            # [5, 15000, 40]

    q1 = np.einsum("rnd,rde->rne", z1, I["Wq1"].astype(np.float32))
    k1 = np.einsum("rnd,rde->rne", z1, I["Wk1"].astype(np.float32))
    v1 = np.einsum("rnd,rde->rne", z1, I["Wv1"].astype(np.float32))
    out1 = _relation_attention(z1, q1, k1, v1, I["Wrel1"].astype(np.float32),
                               H1, C1, N2)
    x2 = out1 + x1 @ I["sw1"].astype(np.float32) + I["sb1"].astype(np.float32)

    # ---- log_softmax
    m = np.max(x2, axis=-1, keepdims=True)
    e = np.exp(x2 - m, dtype=np.float32)
    return (x2 - m - np.log(np.sum(e, axis=-1, keepdims=True))).astype(np.float32)


# revision 14
# speedup vs baseline: 1.1764x; 1.0399x over previous
"""BRGCN (2-layer relational GAT) for Trainium2, 8 NeuronCores.

Strategy (graph/data parallel per sharding hint): layer-0 targets are
sharded contiguously across the 8 cores. The FLOP-dominant dense block --
the per-relation Q/K/V projections of the aggregated messages z
([R=5, 15000, 256] @ [5, 256, 256] x3, ~30 GF) -- runs on the device,
each core owning 1875 target nodes. The irregular, index-dependent
message passing (edge gather, per-(target,relation) softmax,
scatter-add) is prepared around it.

The device kernel is DMA-bound (memory regime): all device I/O is bf16
(fp32 PSUM accumulation), halving HBM traffic vs fp32 -- ~21 MB/core.
Stationary weights are reused across column chunks, PSUM is evacuated
with explicit VectorE copies (casting to bf16), and the four column
chunks of each output row-block are staged into one SBUF tile so each
output DMA is a single large transfer.

Only the first 30000 rows of x and the first 15000 rows of x1 can affect
the output (edge indices are bounded by N1/N2), so everything else is
skipped.
"""
import os
import sys
import numpy as np

for _p in ("/opt/trn_rl_repo", "/root/.axon_site/_ro/trn_rl_repo"):
    if os.path.isdir(_p) and _p not in sys.path:
        sys.path.insert(0, _p)

import ml_dtypes
import concourse.bass as bass
import concourse.bacc as bacc
import concourse.mybir as mybir
import concourse.tile as tile
from concourse.bass_utils import run_bass_kernel_spmd

R = 5
NEG_SLOPE = 0.2
N1 = 30000
N2 = 15000
NCORES = 8
NPC = N2 // NCORES          # 1875 target nodes per core
NPAD = 1880                 # padded to 4 chunks of 470
NCHUNK = 470
HC0 = 256
BF16 = ml_dtypes.bfloat16

LAST_RESULTS = None         # BassKernelResults of the device launch

_compiled = None


def _ensure_ntff_hook():
    """bass_utils' trace path imports antenv.axon_hooks, which this image's
    antenv package lacks. Inject an equivalent in-memory module wired to
    the axon PJRT .so so NTFF profiling (exec_time_ns) works. Returns True
    if the trace path is usable."""
    try:
        import antenv.axon_hooks  # noqa: F401
        return True
    except ImportError:
        pass
    try:
        import types
        import antenv
        from trn_agent_boot.trn_boot import _ntff_profile_via_ctypes
        hook = _ntff_profile_via_ctypes("/opt/axon/libaxon_pjrt.so")
        mod = types.ModuleType("antenv.axon_hooks")
        state = {"hook": hook}
        mod.get_axon_ntff_profile_hook = lambda: state["hook"]
        mod.set_axon_ntff_profile_hook = lambda h: state.update(hook=h)
        sys.modules["antenv.axon_hooks"] = mod
        antenv.axon_hooks = mod
        return hook is not None
    except Exception as e:
        sys.stderr.write(f"[kernel] ntff hook setup failed ({e!r})\n")
        return False


def _build_device_program():
    """Per-core program: for r in 0..4, q/k/v^T = W^T @ z_r^T, all bf16 I/O.

    Inputs  zT  [5, 256, 1880] bf16   (this core's z, transposed, padded)
            wq/wk/wv [5, 256, 256] bf16
    Outputs qT/kT/vT [5, 256, 1880] bf16
    """
    nc = bacc.Bacc("TRN2", target_bir_lowering=False, debug=False,
                   num_devices=NCORES)
    bf16 = mybir.dt.bfloat16
    f32 = mybir.dt.float32

    zT = nc.declare_dram_parameter("zT", [R, HC0, NPAD], bf16, isOutput=False)
    ws = [nc.declare_dram_parameter(n, [R, HC0, HC0], bf16, isOutput=False)
          for n in ("wq", "wk", "wv")]
    outs = [nc.declare_dram_parameter(n, [R, HC0, NPAD], bf16, isOutput=True)
            for n in ("qT", "kT", "vT")]

    with tile.TileContext(nc) as tc:
        with (
            tc.tile_pool(name="zp", bufs=1) as zp,
            tc.tile_pool(name="wp", bufs=1) as wp,
            tc.tile_pool(name="sp", bufs=4) as sp,
            tc.tile_pool(name="tp", bufs=4) as tp,
            tc.tile_pool(name="ps", bufs=2, space="PSUM") as psp,
        ):
            # Few LARGE input DMAs (HW-queue DMAs are completion-serialized
            # by the FIFO sem scheme, ~0.6-1us each): one per (w-tensor,
            # k-chunk) and one per z k-chunk, batched across all relations.
            # Ordered so the first matmul's operands land first.
            wts = {}
            zt = {}

            def load_w(wi, kc):
                t = wp.tile([128, R * HC0], bf16, tag=f"w{wi}_{kc}", name="w")
                nc.sync.dma_start(
                    out=t[:].rearrange("p (r m) -> p r m", r=R),
                    in_=ws[wi][:, kc * 128:(kc + 1) * 128, :]
                        .rearrange("r p m -> p r m"),
                )
                wts[(wi, kc)] = t

            def load_z(kc):
                t = zp.tile([128, R * NPAD], bf16, tag=f"z{kc}", name="z")
                nc.sync.dma_start(
                    out=t[:].rearrange("p (r n) -> p r n", r=R),
                    in_=zT[:, kc * 128:(kc + 1) * 128, :]
                        .rearrange("r p n -> p r n"),
                )
                zt[kc] = t

            def load_z_r(kc, r):
                t = zt[kc]
                nc.sync.dma_start(
                    out=t[:, r * NPAD:(r + 1) * NPAD],
                    in_=zT[r, kc * 128:(kc + 1) * 128, :],
                )

            # allocate z tiles up front, fill per (kc, r) so the first
            # matmul's slice lands within ~2us
            for kc in range(2):
                t = zp.tile([128, R * NPAD], bf16, tag=f"z{kc}", name="z")
                zt[kc] = t
            load_w(0, 0)
            load_z_r(0, 0)
            load_w(0, 1)
            load_z_r(1, 0)
            for r in range(1, R):
                load_z_r(0, r)
                load_z_r(1, r)
            for wi in (1, 2):
                for kc in range(2):
                    load_w(wi, kc)

            # 4 column chunks of 512/512/512/344 (bank-aligned): each matmul
            # writes one PSUM bank; evacuation is ONE DVE copy (banks 0-1)
            # plus ONE ACT copy (banks 2-3) per mc-group -- both engines
            # measure ~same per-op cost, and 2-bank copies amortize the
            # ~275ns per-op overhead.
            CH = [(0, 512), (512, 512), (1024, 512), (1536, 344)]
            for wi, o in enumerate(outs):
                for r in range(R):
                    # stage both mc row-blocks of (wi, r), then store with
                    # ONE DMA from the (idle) SP ring
                    st = sp.tile([128, 2 * NPAD], bf16, tag="stage",
                                 name="st")
                    for mc in range(2):
                        ps = psp.tile([128, NPAD], f32, tag="acc", name="acc")
                        # kc outer / chunk inner: stationary weight reused
                        # across the 4 column chunks
                        for kc in range(2):
                            for c0, cw in CH:
                                nc.tensor.matmul(
                                    out=ps[:, c0:c0 + cw],
                                    lhsT=wts[(wi, kc)][:, r * HC0 + mc * 128:
                                                       r * HC0 + mc * 128 + 128],
                                    rhs=zt[kc][:, r * NPAD + c0:
                                               r * NPAD + c0 + cw],
                                    start=(kc == 0),
                                    stop=(kc == 1),
                                )
                        # alternate which engine takes the wide half so DVE
                        # and ACT see equal work overall
                        lo = st[:, mc * NPAD:mc * NPAD + 1024]
                        hi = st[:, mc * NPAD + 1024:(mc + 1) * NPAD]
                        if mc == 0:
                            nc.vector.tensor_copy(out=lo, in_=ps[:, 0:1024])
                            nc.scalar.copy(out=hi, in_=ps[:, 1024:NPAD])
                        else:
                            nc.scalar.copy(out=lo, in_=ps[:, 0:1024])
                            nc.vector.tensor_copy(out=hi, in_=ps[:, 1024:NPAD])
                    nc.sync.dma_start(
                        out=o[r].rearrange("(mc p) n -> p mc n", mc=2),
                        in_=st[:].rearrange("p (mc n) -> p mc n", mc=2),
                    )
    nc.finalize()   # Bacc.compile(): legalizes multi-sem waits (1/inst on TRN2)
    return nc


def _device_qkv(z):
    """z [R, N2, 256] f32 -> q, k, v [R, N2, 256] via the 8-core kernel."""
    global _compiled, LAST_RESULTS
    if _compiled is None:
        _compiled = _build_device_program()
    nc = _compiled
    zb = z.astype(BF16)
    in_maps = []
    for d in range(NCORES):
        zs = zb[:, d * NPC:(d + 1) * NPC, :]                 # [5, 1875, 256]
        zt = np.zeros((R, HC0, NPAD), dtype=BF16)
        zt[:, :, :NPC] = zs.transpose(0, 2, 1)
        in_maps.append({"zT": zt, "wq": _W[0], "wk": _W[1], "wv": _W[2]})
    res = run_bass_kernel_spmd(
        nc, in_maps, list(range(NCORES)),
        trace=bool(os.environ.get("KERNEL_TRACE")) and _ensure_ntff_hook(),
    )
    LAST_RESULTS = res
    q = np.empty((R, N2, HC0), dtype=np.float32)
    k = np.empty((R, N2, HC0), dtype=np.float32)
    v = np.empty((R, N2, HC0), dtype=np.float32)
    for d in range(NCORES):
        rd = res.results[d]
        sl = slice(d * NPC, (d + 1) * NPC)
        q[:, sl, :] = rd["qT"][:, :, :NPC].transpose(0, 2, 1).astype(np.float32)
        k[:, sl, :] = rd["kT"][:, :, :NPC].transpose(0, 2, 1).astype(np.float32)
        v[:, sl, :] = rd["vT"][:, :, :NPC].transpose(0, 2, 1).astype(np.float32)
    return q, k, v


_W = None


def _seg_softmax_scatter(alpha, xj, seg, nseg, hc):
    """Edge softmax grouped by seg, then weighted scatter-add of xj.

    Sort-by-segment + reduceat: identical math to segment_max/segment_sum
    (empty segments yield zero rows), much faster than np.add.at.
    """
    E, H = alpha.shape
    order = np.argsort(seg, kind="stable")
    seg_s = seg[order]
    alpha_s = alpha[order]
    starts = np.flatnonzero(np.r_[True, seg_s[1:] != seg_s[:-1]])
    uniq = seg_s[starts]
    amax = np.zeros((nseg, H), dtype=np.float32)
    amax[uniq] = np.maximum.reduceat(alpha_s, starts, axis=0)
    ex_s = np.exp(alpha_s - amax[seg_s], dtype=np.float32)
    den = np.zeros((nseg, H), dtype=np.float32)
    den[uniq] = np.add.reduceat(ex_s, starts, axis=0)
    w_s = ex_s / np.maximum(den[seg_s], 1e-16)
    msg_s = (w_s[:, :, None] * xj[order].reshape(E, H, -1)).reshape(E, hc)
    z = np.zeros((nseg, hc), dtype=np.float32)
    z[uniq] = np.add.reduceat(msg_s.astype(np.float32), starts, axis=0)
    return z


def _relation_attention(z, q, k, v, Wrel, heads, outc, N):
    hc = heads * outc
    qh = q.reshape(R, N, heads, outc)
    kh = k.reshape(R, N, heads, outc)
    vh = v.reshape(R, N, heads, outc)
    psi = np.einsum("rnhc,snhc->rsnh", qh, kh).astype(np.float32)
    mask = (psi == 0) & (np.sum(psi, axis=1, keepdims=True) != 0)
    psi_m = np.where(mask, -np.inf, psi)
    pm = np.max(psi_m, axis=1, keepdims=True)
    pe = np.exp(psi_m - pm, dtype=np.float32)
    prob = pe / np.sum(pe, axis=1, keepdims=True)
    delta = np.einsum("rsnh,snhc->rnhc", prob, vh).reshape(R, N, hc)
    return np.einsum("rnd,r->nd", delta, Wrel[:, 0]).astype(np.float32)


def kernel(**inputs):
    global _W
    I = {k: np.asarray(val) for k, val in inputs.items()}
    emb = I["emb"].astype(np.float32)
    nid = I["n_id"].astype(np.int64)
    lni = I["local_node_idx"].astype(np.int64)

    # ---- group_input (only the 30000 rows that matter)
    x = emb[lni[nid[:N1]]]                                   # [30000, 128]

    # ---- layer 0: per-relation GAT over edges with tgt < 15000
    ei0 = I["edge_index0"].astype(np.int64)
    et0 = I["edge_type0"].astype(np.int64)
    keep = ei0[1] < N2
    src, tgt, rel = ei0[0][keep], ei0[1][keep], et0[keep]

    Wj0, Wi0 = I["Wj0"].astype(np.float32), I["Wi0"].astype(np.float32)
    att_j0, att_i0 = I["att_j0"].astype(np.float32), I["att_i0"].astype(np.float32)
    hj = (x @ Wj0).astype(np.float32)                        # [30000, 256]
    hi = (x[:N2] @ Wi0).astype(np.float32)                   # [15000, 256]
    H0, C0 = 4, 64
    xj = hj[src]                                             # [E, 256]
    xi = hi[tgt]
    aj = np.einsum("ehc,ehc->eh", att_j0[rel], xj.reshape(-1, H0, C0))
    ai = np.einsum("ehc,ehc->eh", att_i0[rel], xi.reshape(-1, H0, C0))
    s = (aj + ai).astype(np.float32)
    alpha = np.where(s >= 0, s, NEG_SLOPE * s).astype(np.float32)
    seg = tgt * R + rel
    z = _seg_softmax_scatter(alpha, xj, seg, N2 * R, HC0)
    z = z.reshape(N2, R, HC0).transpose(1, 0, 2)             # [5, 15000, 256]

    # ---- device: per-relation Q/K/V projections (the dominant dense block)
    _W = (np.ascontiguousarray(I["Wq0"].astype(np.float32)).astype(BF16),
          np.ascontiguousarray(I["Wk0"].astype(np.float32)).astype(BF16),
          np.ascontiguousarray(I["Wv0"].astype(np.float32)).astype(BF16))
    try:
        q, k, v = _device_qkv(z)
    except Exception as e:  # device unavailable -> host fallback, stays correct
        sys.stderr.write(f"[kernel] device path failed ({e!r}); host fallback\n")
        W = [w.astype(np.float32) for w in _W]
        q = np.einsum("rnd,rde->rne", z, W[0]).astype(np.float32)
        k = np.einsum("rnd,rde->rne", z, W[1]).astype(np.float32)
        v = np.einsum("rnd,rde->rne", z, W[2]).astype(np.float32)

    out0 = _relation_attention(z, q, k, v, I["Wrel0"].astype(np.float32), H0, C0, N2)
    x1 = out0 + x[:N2] @ I["sw0"].astype(np.float32) + I["sb0"].astype(np.float32)
    x1 = np.maximum(x1, 0.0).astype(np.float32)              # [15000, 256]

    # ---- layer 1 (small: 40-dim), host
    ei1 = I["edge_index1"].astype(np.int64)
    et1 = I["edge_type1"].astype(np.int64)
    src1, tgt1, rel1 = ei1[0], ei1[1], et1
    Wj1, Wi1 = I["Wj1"].astype(np.float32), I["Wi1"].astype(np.float32)
    hj1 = (x1 @ Wj1).astype(np.float32)                      # [15000, 40]
    hi1 = (x1[:N2] @ Wi1).astype(np.float32)
    H1, C1 = 1, 40
    xj1 = hj1[src1]
    xi1 = hi1[tgt1]
    aj1 = np.einsum("ehc,ehc->eh", I["att_j1"].astype(np.float32)[rel1],
                    xj1.reshape(-1, H1, C1))
    ai1 = np.einsum("ehc,ehc->eh", I["att_i1"].astype(np.float32)[rel1],
                    xi1.reshape(-1, H1, C1))
    s1 = (aj1 + ai1).astype(np.float32)
    alpha1 = np.where(s1 >= 0, s1, NEG_SLOPE * s1).astype(np.float32)
    seg1 = tgt1 * R + rel1
    z1 = _seg_softmax_scatter(alpha1, xj1, seg1, N2 * R, C1)
    z1 = z1.reshape(N2, R, C1).transpose(1, 0, 2)            # [5, 15000, 40]

    q1 = np.einsum("rnd,rde->rne", z1, I["Wq1"].astype(np.float32))
    k1 = np.einsum("rnd,rde->rne", z1, I["Wk1"].astype(np.float32))
    v1 = np.einsum("rnd,rde->rne", z1, I["Wv1"].astype(np.float32))
    out1 = _relation_attention(z1, q1, k1, v1, I["Wrel1"].astype(np.float32),
                               H1, C1, N2)
    x2 = out1 + x1 @ I["sw1"].astype(np.float32) + I["sb1"].astype(np.float32)

    # ---- log_softmax
    m = np.max(x2, axis=-1, keepdims=True)
    e = np.exp(x2 - m, dtype=np.float32)
    return (x2 - m - np.log(np.sum(e, axis=-1, keepdims=True))).astype(np.float32)


# revision 15
# speedup vs baseline: 1.4751x; 1.2539x over previous
"""BRGCN (2-layer relational GAT) for Trainium2, 8 NeuronCores.

Strategy (graph/data parallel per sharding hint): layer-0 targets are
sharded contiguously across the 8 cores. The FLOP-dominant dense block --
the per-relation Q/K/V projections of the aggregated messages z
([R=5, 15000, 256] @ [5, 256, 256] x3, ~30 GF) -- runs on the device,
each core owning 1875 target nodes. The irregular, index-dependent
message passing (edge gather, per-(target,relation) softmax,
scatter-add) is prepared around it.

The device kernel is DMA-bound (memory regime): all device I/O is bf16
(fp32 PSUM accumulation), halving HBM traffic vs fp32 -- ~21 MB/core.
Stationary weights are reused across column chunks, PSUM is evacuated
with explicit VectorE copies (casting to bf16), and the four column
chunks of each output row-block are staged into one SBUF tile so each
output DMA is a single large transfer.

Only the first 30000 rows of x and the first 15000 rows of x1 can affect
the output (edge indices are bounded by N1/N2), so everything else is
skipped.
"""
import os
import sys
import numpy as np

for _p in ("/opt/trn_rl_repo", "/root/.axon_site/_ro/trn_rl_repo"):
    if os.path.isdir(_p) and _p not in sys.path:
        sys.path.insert(0, _p)

import ml_dtypes
import concourse.bass as bass
import concourse.bacc as bacc
import concourse.mybir as mybir
import concourse.tile as tile
from concourse.bass_utils import run_bass_kernel_spmd

R = 5
NEG_SLOPE = 0.2
N1 = 30000
N2 = 15000
NCORES = 8
NPC = N2 // NCORES          # 1875 target nodes per core
NPAD = 1880                 # padded to 4 chunks of 470
NCHUNK = 470
HC0 = 256
BF16 = ml_dtypes.bfloat16

LAST_RESULTS = None         # BassKernelResults of the device launch

_compiled = None


def _ensure_ntff_hook():
    """bass_utils' trace path imports antenv.axon_hooks, which this image's
    antenv package lacks. Inject an equivalent in-memory module wired to
    the axon PJRT .so so NTFF profiling (exec_time_ns) works. Returns True
    if the trace path is usable."""
    try:
        import antenv.axon_hooks  # noqa: F401
        return True
    except ImportError:
        pass
    try:
        import types
        import antenv
        from trn_agent_boot.trn_boot import _ntff_profile_via_ctypes
        hook = _ntff_profile_via_ctypes("/opt/axon/libaxon_pjrt.so")
        mod = types.ModuleType("antenv.axon_hooks")
        state = {"hook": hook}
        mod.get_axon_ntff_profile_hook = lambda: state["hook"]
        mod.set_axon_ntff_profile_hook = lambda h: state.update(hook=h)
        sys.modules["antenv.axon_hooks"] = mod
        antenv.axon_hooks = mod
        return hook is not None
    except Exception as e:
        sys.stderr.write(f"[kernel] ntff hook setup failed ({e!r})\n")
        return False


def _build_device_program():
    """Per-core program: for r in 0..4, q/k/v^T = W^T @ z_r^T, all bf16 I/O.

    Inputs  zT  [5, 256, 1880] bf16   (this core's z, transposed, padded)
            wq/wk/wv [5, 256, 256] bf16
    Outputs qT/kT/vT [5, 256, 1880] bf16
    """
    nc = bacc.Bacc("TRN2", target_bir_lowering=False, debug=False,
                   num_devices=NCORES)
    bf16 = mybir.dt.bfloat16
    f32 = mybir.dt.float32

    zT = nc.declare_dram_parameter("zT", [R, HC0, NPAD], bf16, isOutput=False)
    ws = [nc.declare_dram_parameter(n, [R, HC0, HC0], bf16, isOutput=False)
          for n in ("wq", "wk", "wv")]
    outs = [nc.declare_dram_parameter(n, [R, HC0, NPAD], bf16, isOutput=True)
            for n in ("qT", "kT", "vT")]

    with tile.TileContext(nc) as tc:
        with (
            tc.tile_pool(name="zp", bufs=1) as zp,
            tc.tile_pool(name="wp", bufs=1) as wp,
            tc.tile_pool(name="sp", bufs=4) as sp,
            tc.tile_pool(name="tp", bufs=4) as tp,
            tc.tile_pool(name="ps", bufs=2, space="PSUM") as psp,
        ):
            # Few LARGE input DMAs (HW-queue DMAs are completion-serialized
            # by the FIFO sem scheme, ~0.6-1us each): one per (w-tensor,
            # k-chunk) and one per z k-chunk, batched across all relations.
            # Ordered so the first matmul's operands land first.
            wts = {}
            zt = {}

            def load_w(wi, kc):
                t = wp.tile([128, R * HC0], bf16, tag=f"w{wi}_{kc}", name="w")
                nc.sync.dma_start(
                    out=t[:].rearrange("p (r m) -> p r m", r=R),
                    in_=ws[wi][:, kc * 128:(kc + 1) * 128, :]
                        .rearrange("r p m -> p r m"),
                )
                wts[(wi, kc)] = t

            def load_z(kc):
                t = zp.tile([128, R * NPAD], bf16, tag=f"z{kc}", name="z")
                nc.sync.dma_start(
                    out=t[:].rearrange("p (r n) -> p r n", r=R),
                    in_=zT[:, kc * 128:(kc + 1) * 128, :]
                        .rearrange("r p n -> p r n"),
                )
                zt[kc] = t

            def load_z_r(kc, r):
                t = zt[kc]
                nc.sync.dma_start(
                    out=t[:, r * NPAD:(r + 1) * NPAD],
                    in_=zT[r, kc * 128:(kc + 1) * 128, :],
                )

            # allocate z tiles up front, fill per (kc, r) so the first
            # matmul's slice lands within ~2us
            for kc in range(2):
                t = zp.tile([128, R * NPAD], bf16, tag=f"z{kc}", name="z")
                zt[kc] = t
            load_w(0, 0)
            load_z_r(0, 0)
            load_w(0, 1)
            load_z_r(1, 0)
            for r in range(1, R):
                load_z_r(0, r)
                load_z_r(1, r)
            for wi in (1, 2):
                for kc in range(2):
                    load_w(wi, kc)

            # 4 column chunks of 512/512/512/344 (bank-aligned): each matmul
            # writes one PSUM bank; evacuation is ONE DVE copy (banks 0-1)
            # plus ONE ACT copy (banks 2-3) per mc-group -- both engines
            # measure ~same per-op cost, and 2-bank copies amortize the
            # ~275ns per-op overhead.
            CH = [(0, 512), (512, 512), (1024, 512), (1536, 344)]
            for wi, o in enumerate(outs):
                for r in range(R):
                    # stage both mc row-blocks of (wi, r), then store with
                    # ONE DMA from the (idle) SP ring
                    st = sp.tile([128, 2 * NPAD], bf16, tag="stage",
                                 name="st")
                    for mc in range(2):
                        # two independent 2-bank PSUM tiles per group: each
                        # half recycles as soon as ITS copy lands (4-deep
                        # pipelining over the 8 banks), killing the ~0.7us
                        # PE stall at every group boundary
                        pa = psp.tile([128, 1024], f32, tag="accA", name="pa")
                        pb = psp.tile([128, 856], f32, tag="accB", name="pb")
                        # kc outer / chunk inner: stationary weight reused
                        # across the 4 column chunks
                        for kc in range(2):
                            for c0, cw in CH:
                                dst = (pa[:, c0:c0 + cw] if c0 < 1024
                                       else pb[:, c0 - 1024:c0 - 1024 + cw])
                                nc.tensor.matmul(
                                    out=dst,
                                    lhsT=wts[(wi, kc)][:, r * HC0 + mc * 128:
                                                       r * HC0 + mc * 128 + 128],
                                    rhs=zt[kc][:, r * NPAD + c0:
                                               r * NPAD + c0 + cw],
                                    start=(kc == 0),
                                    stop=(kc == 1),
                                )
                        nc.vector.tensor_copy(
                            out=st[:, mc * NPAD:mc * NPAD + 1024],
                            in_=pa[:])
                        nc.scalar.copy(
                            out=st[:, mc * NPAD + 1024:(mc + 1) * NPAD],
                            in_=pb[:])
                    nc.sync.dma_start(
                        out=o[r].rearrange("(mc p) n -> p mc n", mc=2),
                        in_=st[:].rearrange("p (mc n) -> p mc n", mc=2),
                    )
    nc.finalize()   # Bacc.compile(): legalizes multi-sem waits (1/inst on TRN2)
    return nc


def _device_qkv(z):
    """z [R, N2, 256] f32 -> q, k, v [R, N2, 256] via the 8-core kernel."""
    global _compiled, LAST_RESULTS
    if _compiled is None:
        _compiled = _build_device_program()
    nc = _compiled
    zb = z.astype(BF16)
    in_maps = []
    for d in range(NCORES):
        zs = zb[:, d * NPC:(d + 1) * NPC, :]                 # [5, 1875, 256]
        zt = np.zeros((R, HC0, NPAD), dtype=BF16)
        zt[:, :, :NPC] = zs.transpose(0, 2, 1)
        in_maps.append({"zT": zt, "wq": _W[0], "wk": _W[1], "wv": _W[2]})
    res = run_bass_kernel_spmd(
        nc, in_maps, list(range(NCORES)),
        trace=bool(os.environ.get("KERNEL_TRACE")) and _ensure_ntff_hook(),
    )
    LAST_RESULTS = res
    q = np.empty((R, N2, HC0), dtype=np.float32)
    k = np.empty((R, N2, HC0), dtype=np.float32)
    v = np.empty((R, N2, HC0), dtype=np.float32)
    for d in range(NCORES):
        rd = res.results[d]
        sl = slice(d * NPC, (d + 1) * NPC)
        q[:, sl, :] = rd["qT"][:, :, :NPC].transpose(0, 2, 1).astype(np.float32)
        k[:, sl, :] = rd["kT"][:, :, :NPC].transpose(0, 2, 1).astype(np.float32)
        v[:, sl, :] = rd["vT"][:, :, :NPC].transpose(0, 2, 1).astype(np.float32)
    return q, k, v


_W = None


def _seg_softmax_scatter(alpha, xj, seg, nseg, hc):
    """Edge softmax grouped by seg, then weighted scatter-add of xj.

    Sort-by-segment + reduceat: identical math to segment_max/segment_sum
    (empty segments yield zero rows), much faster than np.add.at.
    """
    E, H = alpha.shape
    order = np.argsort(seg, kind="stable")
    seg_s = seg[order]
    alpha_s = alpha[order]
    starts = np.flatnonzero(np.r_[True, seg_s[1:] != seg_s[:-1]])
    uniq = seg_s[starts]
    amax = np.zeros((nseg, H), dtype=np.float32)
    amax[uniq] = np.maximum.reduceat(alpha_s, starts, axis=0)
    ex_s = np.exp(alpha_s - amax[seg_s], dtype=np.float32)
    den = np.zeros((nseg, H), dtype=np.float32)
    den[uniq] = np.add.reduceat(ex_s, starts, axis=0)
    w_s = ex_s / np.maximum(den[seg_s], 1e-16)
    msg_s = (w_s[:, :, None] * xj[order].reshape(E, H, -1)).reshape(E, hc)
    z = np.zeros((nseg, hc), dtype=np.float32)
    z[uniq] = np.add.reduceat(msg_s.astype(np.float32), starts, axis=0)
    return z


def _relation_attention(z, q, k, v, Wrel, heads, outc, N):
    hc = heads * outc
    qh = q.reshape(R, N, heads, outc)
    kh = k.reshape(R, N, heads, outc)
    vh = v.reshape(R, N, heads, outc)
    psi = np.einsum("rnhc,snhc->rsnh", qh, kh).astype(np.float32)
    mask = (psi == 0) & (np.sum(psi, axis=1, keepdims=True) != 0)
    psi_m = np.where(mask, -np.inf, psi)
    pm = np.max(psi_m, axis=1, keepdims=True)
    pe = np.exp(psi_m - pm, dtype=np.float32)
    prob = pe / np.sum(pe, axis=1, keepdims=True)
    delta = np.einsum("rsnh,snhc->rnhc", prob, vh).reshape(R, N, hc)
    return np.einsum("rnd,r->nd", delta, Wrel[:, 0]).astype(np.float32)


def kernel(**inputs):
    global _W
    I = {k: np.asarray(val) for k, val in inputs.items()}
    emb = I["emb"].astype(np.float32)
    nid = I["n_id"].astype(np.int64)
    lni = I["local_node_idx"].astype(np.int64)

    # ---- group_input (only the 30000 rows that matter)
    x = emb[lni[nid[:N1]]]                                   # [30000, 128]

    # ---- layer 0: per-relation GAT over edges with tgt < 15000
    ei0 = I["edge_index0"].astype(np.int64)
    et0 = I["edge_type0"].astype(np.int64)
    keep = ei0[1] < N2
    src, tgt, rel = ei0[0][keep], ei0[1][keep], et0[keep]

    Wj0, Wi0 = I["Wj0"].astype(np.float32), I["Wi0"].astype(np.float32)
    att_j0, att_i0 = I["att_j0"].astype(np.float32), I["att_i0"].astype(np.float32)
    hj = (x @ Wj0).astype(np.float32)                        # [30000, 256]
    hi = (x[:N2] @ Wi0).astype(np.float32)                   # [15000, 256]
    H0, C0 = 4, 64
    xj = hj[src]                                             # [E, 256]
    xi = hi[tgt]
    aj = np.einsum("ehc,ehc->eh", att_j0[rel], xj.reshape(-1, H0, C0))
    ai = np.einsum("ehc,ehc->eh", att_i0[rel], xi.reshape(-1, H0, C0))
    s = (aj + ai).astype(np.float32)
    alpha = np.where(s >= 0, s, NEG_SLOPE * s).astype(np.float32)
    seg = tgt * R + rel
    z = _seg_softmax_scatter(alpha, xj, seg, N2 * R, HC0)
    z = z.reshape(N2, R, HC0).transpose(1, 0, 2)             # [5, 15000, 256]

    # ---- device: per-relation Q/K/V projections (the dominant dense block)
    _W = (np.ascontiguousarray(I["Wq0"].astype(np.float32)).astype(BF16),
          np.ascontiguousarray(I["Wk0"].astype(np.float32)).astype(BF16),
          np.ascontiguousarray(I["Wv0"].astype(np.float32)).astype(BF16))
    try:
        q, k, v = _device_qkv(z)
    except Exception as e:  # device unavailable -> host fallback, stays correct
        sys.stderr.write(f"[kernel] device path failed ({e!r}); host fallback\n")
        W = [w.astype(np.float32) for w in _W]
        q = np.einsum("rnd,rde->rne", z, W[0]).astype(np.float32)
        k = np.einsum("rnd,rde->rne", z, W[1]).astype(np.float32)
        v = np.einsum("rnd,rde->rne", z, W[2]).astype(np.float32)

    out0 = _relation_attention(z, q, k, v, I["Wrel0"].astype(np.float32), H0, C0, N2)
    x1 = out0 + x[:N2] @ I["sw0"].astype(np.float32) + I["sb0"].astype(np.float32)
    x1 = np.maximum(x1, 0.0).astype(np.float32)              # [15000, 256]

    # ---- layer 1 (small: 40-dim), host
    ei1 = I["edge_index1"].astype(np.int64)
    et1 = I["edge_type1"].astype(np.int64)
    src1, tgt1, rel1 = ei1[0], ei1[1], et1
    Wj1, Wi1 = I["Wj1"].astype(np.float32), I["Wi1"].astype(np.float32)
    hj1 = (x1 @ Wj1).astype(np.float32)                      # [15000, 40]
    hi1 = (x1[:N2] @ Wi1).astype(np.float32)
    H1, C1 = 1, 40
    xj1 = hj1[src1]
    xi1 = hi1[tgt1]
    aj1 = np.einsum("ehc,ehc->eh", I["att_j1"].astype(np.float32)[rel1],
                    xj1.reshape(-1, H1, C1))
    ai1 = np.einsum("ehc,ehc->eh", I["att_i1"].astype(np.float32)[rel1],
                    xi1.reshape(-1, H1, C1))
    s1 = (aj1 + ai1).astype(np.float32)
    alpha1 = np.where(s1 >= 0, s1, NEG_SLOPE * s1).astype(np.float32)
    seg1 = tgt1 * R + rel1
    z1 = _seg_softmax_scatter(alpha1, xj1, seg1, N2 * R, C1)
    z1 = z1.reshape(N2, R, C1).transpose(1, 0, 2)            # [5, 15000, 40]

    q1 = np.einsum("rnd,rde->rne", z1, I["Wq1"].astype(np.float32))
    k1 = np.einsum("rnd,rde->rne", z1, I["Wk1"].astype(np.float32))
    v1 = np.einsum("rnd,rde->rne", z1, I["Wv1"].astype(np.float32))
    out1 = _relation_attention(z1, q1, k1, v1, I["Wrel1"].astype(np.float32),
                               H1, C1, N2)
    x2 = out1 + x1 @ I["sw1"].astype(np.float32) + I["sb1"].astype(np.float32)

    # ---- log_softmax
    m = np.max(x2, axis=-1, keepdims=True)
    e = np.exp(x2 - m, dtype=np.float32)
    return (x2 - m - np.log(np.sum(e, axis=-1, keepdims=True))).astype(np.float32)


# revision 16
# speedup vs baseline: 1.5184x; 1.0294x over previous
"""BRGCN (2-layer relational GAT) for Trainium2, 8 NeuronCores.

Strategy (graph/data parallel per sharding hint): layer-0 targets are
sharded contiguously across the 8 cores. The FLOP-dominant dense block --
the per-relation Q/K/V projections of the aggregated messages z
([R=5, 15000, 256] @ [5, 256, 256] x3, ~30 GF) -- runs on the device,
each core owning 1875 target nodes. The irregular, index-dependent
message passing (edge gather, per-(target,relation) softmax,
scatter-add) is prepared around it.

The device kernel is DMA-bound (memory regime): all device I/O is bf16
(fp32 PSUM accumulation), halving HBM traffic vs fp32 -- ~21 MB/core.
Stationary weights are reused across column chunks, PSUM is evacuated
with explicit VectorE copies (casting to bf16), and the four column
chunks of each output row-block are staged into one SBUF tile so each
output DMA is a single large transfer.

Only the first 30000 rows of x and the first 15000 rows of x1 can affect
the output (edge indices are bounded by N1/N2), so everything else is
skipped.
"""
import os
import sys
import numpy as np

for _p in ("/opt/trn_rl_repo", "/root/.axon_site/_ro/trn_rl_repo"):
    if os.path.isdir(_p) and _p not in sys.path:
        sys.path.insert(0, _p)

import ml_dtypes
import concourse.bass as bass
import concourse.bacc as bacc
import concourse.mybir as mybir
import concourse.tile as tile
from concourse.bass_utils import run_bass_kernel_spmd

R = 5
NEG_SLOPE = 0.2
N1 = 30000
N2 = 15000
NCORES = 8
NPC = N2 // NCORES          # 1875 target nodes per core
NPAD = 1880                 # padded to 4 chunks of 470
NCHUNK = 470
HC0 = 256
BF16 = ml_dtypes.bfloat16

FP8 = ml_dtypes.float8_e4m3
SZ, SW = 32.0, 16.0         # fp8 pre-scales for z and the weights
LAST_RESULTS = None         # BassKernelResults of the device launch

_compiled = None


def _ensure_ntff_hook():
    """bass_utils' trace path imports antenv.axon_hooks, which this image's
    antenv package lacks. Inject an equivalent in-memory module wired to
    the axon PJRT .so so NTFF profiling (exec_time_ns) works. Returns True
    if the trace path is usable."""
    try:
        import antenv.axon_hooks  # noqa: F401
        return True
    except ImportError:
        pass
    try:
        import types
        import antenv
        from trn_agent_boot.trn_boot import _ntff_profile_via_ctypes
        hook = _ntff_profile_via_ctypes("/opt/axon/libaxon_pjrt.so")
        mod = types.ModuleType("antenv.axon_hooks")
        state = {"hook": hook}
        mod.get_axon_ntff_profile_hook = lambda: state["hook"]
        mod.set_axon_ntff_profile_hook = lambda h: state.update(hook=h)
        sys.modules["antenv.axon_hooks"] = mod
        antenv.axon_hooks = mod
        return hook is not None
    except Exception as e:
        sys.stderr.write(f"[kernel] ntff hook setup failed ({e!r})\n")
        return False


def _build_device_program():
    """Per-core program: for r in 0..4, q/k/v^T = W^T @ z_r^T, all bf16 I/O.

    Inputs  zT  [5, 256, 1880] bf16   (this core's z, transposed, padded)
            wq/wk/wv [5, 256, 256] bf16
    Outputs qT/kT/vT [5, 256, 1880] bf16
    """
    nc = bacc.Bacc("TRN2", target_bir_lowering=False, debug=False,
                   num_devices=NCORES)
    bf16 = mybir.dt.bfloat16
    fp8 = mybir.dt.float8e4
    f32 = mybir.dt.float32

    zT = nc.declare_dram_parameter("zT", [R, HC0, NPAD], fp8, isOutput=False)
    ws = [nc.declare_dram_parameter(n, [R, HC0, HC0], fp8, isOutput=False)
          for n in ("wq", "wk", "wv")]
    outs = [nc.declare_dram_parameter(n, [R, HC0, NPAD], bf16, isOutput=True)
            for n in ("qT", "kT", "vT")]

    with tile.TileContext(nc) as tc:
        with (
            tc.tile_pool(name="zp", bufs=1) as zp,
            tc.tile_pool(name="wp", bufs=1) as wp,
            tc.tile_pool(name="sp", bufs=4) as sp,
            tc.tile_pool(name="tp", bufs=4) as tp,
            tc.tile_pool(name="ps", bufs=2, space="PSUM") as psp,
        ):
            # Few LARGE input DMAs (HW-queue DMAs are completion-serialized
            # by the FIFO sem scheme, ~0.6-1us each): one per (w-tensor,
            # k-chunk) and one per z k-chunk, batched across all relations.
            # Ordered so the first matmul's operands land first.
            wts = {}
            zt = {}

            def load_w(wi, kc):
                if wi not in wts:
                    wts[wi] = wp.tile([128, 2 * R * HC0], fp8,
                                      tag=f"w{wi}", name="w")
                t = wts[wi]
                nc.sync.dma_start(
                    out=t[:, kc * R * HC0:(kc + 1) * R * HC0]
                        .rearrange("p (r m) -> p r m", r=R),
                    in_=ws[wi][:, kc * 128:(kc + 1) * 128, :]
                        .rearrange("r p m -> p r m"),
                )

            def load_z(kc):
                t = zp.tile([128, R * NPAD], bf16, tag=f"z{kc}", name="z")
                nc.sync.dma_start(
                    out=t[:].rearrange("p (r n) -> p r n", r=R),
                    in_=zT[:, kc * 128:(kc + 1) * 128, :]
                        .rearrange("r p n -> p r n"),
                )
                zt[kc] = t

            def load_z_r(kc, r):
                t = zt[0]
                nc.sync.dma_start(
                    out=t[:, (kc * R + r) * NPAD:(kc * R + r + 1) * NPAD],
                    in_=zT[r, kc * 128:(kc + 1) * 128, :],
                )

            # one z tile spanning both k-chunks, filled per (kc, r) so the
            # first matmul's slices land within ~2us
            zt[0] = zp.tile([128, 2 * R * NPAD], fp8, tag="z", name="z")
            load_w(0, 0)
            load_z_r(0, 0)
            load_w(0, 1)
            load_z_r(1, 0)
            for r in range(1, R):
                load_z_r(0, r)
                load_z_r(1, r)
            for wi in (1, 2):
                for kc in range(2):
                    load_w(wi, kc)

            # 4 column chunks of 512/512/512/344 (bank-aligned): each matmul
            # writes one PSUM bank; evacuation is ONE DVE copy (banks 0-1)
            # plus ONE ACT copy (banks 2-3) per mc-group -- both engines
            # measure ~same per-op cost, and 2-bank copies amortize the
            # ~275ns per-op overhead.
            CH = [(0, 512), (512, 512), (1024, 512), (1536, 344)]
            for wi, o in enumerate(outs):
                for r in range(R):
                    # stage both mc row-blocks of (wi, r), then store with
                    # ONE DMA from the (idle) SP ring
                    st = sp.tile([128, 2 * NPAD], bf16, tag="stage",
                                 name="st")
                    for mc in range(2):
                        # two independent 2-bank PSUM tiles per group: each
                        # half recycles as soon as ITS copy lands (4-deep
                        # pipelining over the 8 banks), killing the ~0.7us
                        # PE stall at every group boundary
                        pa = psp.tile([128, 1024], f32, tag="accA", name="pa")
                        pb = psp.tile([128, 856], f32, tag="accB", name="pb")
                        wv3 = wts[wi][:].rearrange(
                            "p (two r m) -> p two r m", two=2, r=R)
                        zv3 = zt[0][:].rearrange(
                            "p (two r n) -> p two r n", two=2, r=R)
                        for c0, cw in CH:
                            dst = (pa[:, c0:c0 + cw] if c0 < 1024
                                   else pb[:, c0 - 1024:c0 - 1024 + cw])
                            nc.tensor.matmul(
                                out=dst,
                                lhsT=wv3[:, :, r, mc * 128:mc * 128 + 128],
                                rhs=zv3[:, :, r, c0:c0 + cw],
                                start=True,
                                stop=True,
                                perf_mode=mybir.MatmulPerfMode.DoubleRow,
                            )
                        nc.vector.tensor_copy(
                            out=st[:, mc * NPAD:mc * NPAD + 1024],
                            in_=pa[:])
                        nc.scalar.copy(
                            out=st[:, mc * NPAD + 1024:(mc + 1) * NPAD],
                            in_=pb[:])
                    nc.sync.dma_start(
                        out=o[r].rearrange("(mc p) n -> p mc n", mc=2),
                        in_=st[:].rearrange("p (mc n) -> p mc n", mc=2),
                    )
    nc.finalize()   # Bacc.compile(): legalizes multi-sem waits (1/inst on TRN2)
    return nc


def _device_qkv(z):
    """z [R, N2, 256] f32 -> q, k, v [R, N2, 256] via the 8-core kernel."""
    global _compiled, LAST_RESULTS
    if _compiled is None:
        _compiled = _build_device_program()
    nc = _compiled
    zb = (z * SZ).astype(FP8)
    in_maps = []
    for d in range(NCORES):
        zs = zb[:, d * NPC:(d + 1) * NPC, :]                 # [5, 1875, 256]
        zt = np.zeros((R, HC0, NPAD), dtype=FP8)
        zt[:, :, :NPC] = zs.transpose(0, 2, 1)
        in_maps.append({"zT": zt, "wq": _W[0], "wk": _W[1], "wv": _W[2]})
    res = run_bass_kernel_spmd(
        nc, in_maps, list(range(NCORES)),
        trace=bool(os.environ.get("KERNEL_TRACE")) and _ensure_ntff_hook(),
    )
    LAST_RESULTS = res
    q = np.empty((R, N2, HC0), dtype=np.float32)
    k = np.empty((R, N2, HC0), dtype=np.float32)
    v = np.empty((R, N2, HC0), dtype=np.float32)
    for d in range(NCORES):
        rd = res.results[d]
        sl = slice(d * NPC, (d + 1) * NPC)
        q[:, sl, :] = rd["qT"][:, :, :NPC].transpose(0, 2, 1).astype(np.float32)
        k[:, sl, :] = rd["kT"][:, :, :NPC].transpose(0, 2, 1).astype(np.float32)
        v[:, sl, :] = rd["vT"][:, :, :NPC].transpose(0, 2, 1).astype(np.float32)
    inv = np.float32(1.0 / (SZ * SW))
    return q * inv, k * inv, v * inv


_W = None


def _seg_softmax_scatter(alpha, xj, seg, nseg, hc):
    """Edge softmax grouped by seg, then weighted scatter-add of xj.

    Sort-by-segment + reduceat: identical math to segment_max/segment_sum
    (empty segments yield zero rows), much faster than np.add.at.
    """
    E, H = alpha.shape
    order = np.argsort(seg, kind="stable")
    seg_s = seg[order]
    alpha_s = alpha[order]
    starts = np.flatnonzero(np.r_[True, seg_s[1:] != seg_s[:-1]])
    uniq = seg_s[starts]
    amax = np.zeros((nseg, H), dtype=np.float32)
    amax[uniq] = np.maximum.reduceat(alpha_s, starts, axis=0)
    ex_s = np.exp(alpha_s - amax[seg_s], dtype=np.float32)
    den = np.zeros((nseg, H), dtype=np.float32)
    den[uniq] = np.add.reduceat(ex_s, starts, axis=0)
    w_s = ex_s / np.maximum(den[seg_s], 1e-16)
    msg_s = (w_s[:, :, None] * xj[order].reshape(E, H, -1)).reshape(E, hc)
    z = np.zeros((nseg, hc), dtype=np.float32)
    z[uniq] = np.add.reduceat(msg_s.astype(np.float32), starts, axis=0)
    return z


def _relation_attention(z, q, k, v, Wrel, heads, outc, N):
    hc = heads * outc
    qh = q.reshape(R, N, heads, outc)
    kh = k.reshape(R, N, heads, outc)
    vh = v.reshape(R, N, heads, outc)
    psi = np.einsum("rnhc,snhc->rsnh", qh, kh).astype(np.float32)
    mask = (psi == 0) & (np.sum(psi, axis=1, keepdims=True) != 0)
    psi_m = np.where(mask, -np.inf, psi)
    pm = np.max(psi_m, axis=1, keepdims=True)
    pe = np.exp(psi_m - pm, dtype=np.float32)
    prob = pe / np.sum(pe, axis=1, keepdims=True)
    delta = np.einsum("rsnh,snhc->rnhc", prob, vh).reshape(R, N, hc)
    return np.einsum("rnd,r->nd", delta, Wrel[:, 0]).astype(np.float32)


def kernel(**inputs):
    global _W
    I = {k: np.asarray(val) for k, val in inputs.items()}
    emb = I["emb"].astype(np.float32)
    nid = I["n_id"].astype(np.int64)
    lni = I["local_node_idx"].astype(np.int64)

    # ---- group_input (only the 30000 rows that matter)
    x = emb[lni[nid[:N1]]]                                   # [30000, 128]

    # ---- layer 0: per-relation GAT over edges with tgt < 15000
    ei0 = I["edge_index0"].astype(np.int64)
    et0 = I["edge_type0"].astype(np.int64)
    keep = ei0[1] < N2
    src, tgt, rel = ei0[0][keep], ei0[1][keep], et0[keep]

    Wj0, Wi0 = I["Wj0"].astype(np.float32), I["Wi0"].astype(np.float32)
    att_j0, att_i0 = I["att_j0"].astype(np.float32), I["att_i0"].astype(np.float32)
    hj = (x @ Wj0).astype(np.float32)                        # [30000, 256]
    hi = (x[:N2] @ Wi0).astype(np.float32)                   # [15000, 256]
    H0, C0 = 4, 64
    xj = hj[src]                                             # [E, 256]
    xi = hi[tgt]
    aj = np.einsum("ehc,ehc->eh", att_j0[rel], xj.reshape(-1, H0, C0))
    ai = np.einsum("ehc,ehc->eh", att_i0[rel], xi.reshape(-1, H0, C0))
    s = (aj + ai).astype(np.float32)
    alpha = np.where(s >= 0, s, NEG_SLOPE * s).astype(np.float32)
    seg = tgt * R + rel
    z = _seg_softmax_scatter(alpha, xj, seg, N2 * R, HC0)
    z = z.reshape(N2, R, HC0).transpose(1, 0, 2)             # [5, 15000, 256]

    # ---- device: per-relation Q/K/V projections (the dominant dense block)
    _W = (np.ascontiguousarray(I["Wq0"].astype(np.float32) * SW).astype(FP8),
          np.ascontiguousarray(I["Wk0"].astype(np.float32) * SW).astype(FP8),
          np.ascontiguousarray(I["Wv0"].astype(np.float32) * SW).astype(FP8))
    try:
        q, k, v = _device_qkv(z)
    except Exception as e:  # device unavailable -> host fallback, stays correct
        sys.stderr.write(f"[kernel] device path failed ({e!r}); host fallback\n")
        W = [w.astype(np.float32) / SW for w in _W]
        q = np.einsum("rnd,rde->rne", z, W[0]).astype(np.float32)
        k = np.einsum("rnd,rde->rne", z, W[1]).astype(np.float32)
        v = np.einsum("rnd,rde->rne", z, W[2]).astype(np.float32)

    out0 = _relation_attention(z, q, k, v, I["Wrel0"].astype(np.float32), H0, C0, N2)
    x1 = out0 + x[:N2] @ I["sw0"].astype(np.float32) + I["sb0"].astype(np.float32)
    x1 = np.maximum(x1, 0.0).astype(np.float32)              # [15000, 256]

    # ---- layer 1 (small: 40-dim), host
    ei1 = I["edge_index1"].astype(np.int64)
    et1 = I["edge_type1"].astype(np.int64)
    src1, tgt1, rel1 = ei1[0], ei1[1], et1
    Wj1, Wi1 = I["Wj1"].astype(np.float32), I["Wi1"].astype(np.float32)
    hj1 = (x1 @ Wj1).astype(np.float32)                      # [15000, 40]
    hi1 = (x1[:N2] @ Wi1).astype(np.float32)
    H1, C1 = 1, 40
    xj1 = hj1[src1]
    xi1 = hi1[tgt1]
    aj1 = np.einsum("ehc,ehc->eh", I["att_j1"].astype(np.float32)[rel1],
                    xj1.reshape(-1, H1, C1))
    ai1 = np.einsum("ehc,ehc->eh", I["att_i1"].astype(np.float32)[rel1],
                    xi1.reshape(-1, H1, C1))
    s1 = (aj1 + ai1).astype(np.float32)
    alpha1 = np.where(s1 >= 0, s1, NEG_SLOPE * s1).astype(np.float32)
    seg1 = tgt1 * R + rel1
    z1 = _seg_softmax_scatter(alpha1, xj1, seg1, N2 * R, C1)
    z1 = z1.reshape(N2, R, C1).transpose(1, 0, 2)            # [5, 15000, 40]

    q1 = np.einsum("rnd,rde->rne", z1, I["Wq1"].astype(np.float32))
    k1 = np.einsum("rnd,rde->rne", z1, I["Wk1"].astype(np.float32))
    v1 = np.einsum("rnd,rde->rne", z1, I["Wv1"].astype(np.float32))
    out1 = _relation_attention(z1, q1, k1, v1, I["Wrel1"].astype(np.float32),
                               H1, C1, N2)
    x2 = out1 + x1 @ I["sw1"].astype(np.float32) + I["sb1"].astype(np.float32)

    # ---- log_softmax
    m = np.max(x2, axis=-1, keepdims=True)
    e = np.exp(x2 - m, dtype=np.float32)
    return (x2 - m - np.log(np.sum(e, axis=-1, keepdims=True))).astype(np.float32)


# revision 17
# speedup vs baseline: 1.6102x; 1.0605x over previous
"""BRGCN (2-layer relational GAT) for Trainium2, 8 NeuronCores.

Strategy (graph/data parallel per sharding hint): layer-0 targets are
sharded contiguously across the 8 cores. The FLOP-dominant dense block --
the per-relation Q/K/V projections of the aggregated messages z
([R=5, 15000, 256] @ [5, 256, 256] x3, ~30 GF) -- runs on the device,
each core owning 1875 target nodes. The irregular, index-dependent
message passing (edge gather, per-(target,relation) softmax,
scatter-add) is prepared around it.

The device kernel is DMA-bound (memory regime): all device I/O is bf16
(fp32 PSUM accumulation), halving HBM traffic vs fp32 -- ~21 MB/core.
Stationary weights are reused across column chunks, PSUM is evacuated
with explicit VectorE copies (casting to bf16), and the four column
chunks of each output row-block are staged into one SBUF tile so each
output DMA is a single large transfer.

Only the first 30000 rows of x and the first 15000 rows of x1 can affect
the output (edge indices are bounded by N1/N2), so everything else is
skipped.
"""
import os
import sys
import numpy as np

for _p in ("/opt/trn_rl_repo", "/root/.axon_site/_ro/trn_rl_repo"):
    if os.path.isdir(_p) and _p not in sys.path:
        sys.path.insert(0, _p)

import ml_dtypes
import concourse.bass as bass
import concourse.bacc as bacc
import concourse.mybir as mybir
import concourse.tile as tile
from concourse.bass_utils import run_bass_kernel_spmd

R = 5
NEG_SLOPE = 0.2
N1 = 30000
N2 = 15000
NCORES = 8
NPC = N2 // NCORES          # 1875 target nodes per core
NPAD = 1880                 # padded to 4 chunks of 470
NCHUNK = 470
HC0 = 256
BF16 = ml_dtypes.bfloat16

FP8 = ml_dtypes.float8_e4m3
SZ, SW = 32.0, 16.0         # fp8 pre-scales for z and the weights
LAST_RESULTS = None         # BassKernelResults of the device launch

_compiled = None


def _ensure_ntff_hook():
    """bass_utils' trace path imports antenv.axon_hooks, which this image's
    antenv package lacks. Inject an equivalent in-memory module wired to
    the axon PJRT .so so NTFF profiling (exec_time_ns) works. Returns True
    if the trace path is usable."""
    try:
        import antenv.axon_hooks  # noqa: F401
        return True
    except ImportError:
        pass
    try:
        import types
        import antenv
        from trn_agent_boot.trn_boot import _ntff_profile_via_ctypes
        hook = _ntff_profile_via_ctypes("/opt/axon/libaxon_pjrt.so")
        mod = types.ModuleType("antenv.axon_hooks")
        state = {"hook": hook}
        mod.get_axon_ntff_profile_hook = lambda: state["hook"]
        mod.set_axon_ntff_profile_hook = lambda h: state.update(hook=h)
        sys.modules["antenv.axon_hooks"] = mod
        antenv.axon_hooks = mod
        return hook is not None
    except Exception as e:
        sys.stderr.write(f"[kernel] ntff hook setup failed ({e!r})\n")
        return False


def _build_device_program():
    """Per-core program: for r in 0..4, q/k/v^T = W^T @ z_r^T, all bf16 I/O.

    Inputs  zT  [5, 256, 1880] bf16   (this core's z, transposed, padded)
            wq/wk/wv [5, 256, 256] bf16
    Outputs qT/kT/vT [5, 256, 1880] bf16
    """
    nc = bacc.Bacc("TRN2", target_bir_lowering=False, debug=False,
                   num_devices=NCORES)
    bf16 = mybir.dt.bfloat16
    fp8 = mybir.dt.float8e4
    f32 = mybir.dt.float32

    zT = nc.declare_dram_parameter("zT", [R, HC0, NPAD], fp8, isOutput=False)
    ws = [nc.declare_dram_parameter(n, [R, HC0, HC0], fp8, isOutput=False)
          for n in ("wq", "wk", "wv")]
    outs = [nc.declare_dram_parameter(n, [R, HC0, NPAD], bf16, isOutput=True)
            for n in ("qT", "kT", "vT")]

    with tile.TileContext(nc) as tc:
        with (
            tc.tile_pool(name="zp", bufs=1) as zp,
            tc.tile_pool(name="wp", bufs=1) as wp,
            tc.tile_pool(name="sp", bufs=4) as sp,
            tc.tile_pool(name="tp", bufs=4) as tp,
            tc.tile_pool(name="ps", bufs=2, space="PSUM") as psp,
        ):
            # Few LARGE input DMAs (HW-queue DMAs are completion-serialized
            # by the FIFO sem scheme, ~0.6-1us each): one per (w-tensor,
            # k-chunk) and one per z k-chunk, batched across all relations.
            # Ordered so the first matmul's operands land first.
            wts = {}
            zt = {}

            def load_w(wi, kc):
                if wi not in wts:
                    wts[wi] = wp.tile([128, 2 * R * HC0], fp8,
                                      tag=f"w{wi}", name="w")
                t = wts[wi]
                nc.sync.dma_start(
                    out=t[:, kc * R * HC0:(kc + 1) * R * HC0]
                        .rearrange("p (r m) -> p r m", r=R),
                    in_=ws[wi][:, kc * 128:(kc + 1) * 128, :]
                        .rearrange("r p m -> p r m"),
                )

            def load_z(kc):
                t = zp.tile([128, R * NPAD], bf16, tag=f"z{kc}", name="z")
                nc.sync.dma_start(
                    out=t[:].rearrange("p (r n) -> p r n", r=R),
                    in_=zT[:, kc * 128:(kc + 1) * 128, :]
                        .rearrange("r p n -> p r n"),
                )
                zt[kc] = t

            def load_z_r(kc, r):
                t = zt[0]
                nc.sync.dma_start(
                    out=t[:, (kc * R + r) * NPAD:(kc * R + r + 1) * NPAD],
                    in_=zT[r, kc * 128:(kc + 1) * 128, :],
                )

            # one z tile spanning both k-chunks, filled per (kc, r) so the
            # first matmul's slices land within ~2us
            zt[0] = zp.tile([128, 2 * R * NPAD], fp8, tag="z", name="z")
            load_w(0, 0)
            load_z_r(0, 0)
            load_w(0, 1)
            load_z_r(1, 0)
            for r in range(1, R):
                load_z_r(0, r)
                load_z_r(1, r)
            for wi in (1, 2):
                for kc in range(2):
                    load_w(wi, kc)

            # 4 column chunks of 512/512/512/344 (bank-aligned): each matmul
            # writes one PSUM bank; evacuation is ONE DVE copy (banks 0-1)
            # plus ONE ACT copy (banks 2-3) per mc-group -- both engines
            # measure ~same per-op cost, and 2-bank copies amortize the
            # ~275ns per-op overhead.
            CH = [(0, 512), (512, 512), (1024, 512), (1536, 344)]
            for wi, o in enumerate(outs):
                for r in range(R):
                    # stage both mc row-blocks of (wi, r), then store with
                    # ONE DMA from the (idle) SP ring
                    st = sp.tile([128, 2 * NPAD], bf16, tag="stage",
                                 name="st")
                    for mc in range(2):
                        # two independent 2-bank PSUM tiles per group: each
                        # half recycles as soon as ITS copy lands (4-deep
                        # pipelining over the 8 banks), killing the ~0.7us
                        # PE stall at every group boundary
                        pa = psp.tile([128, 1024], f32, tag="accA", name="pa")
                        pb = psp.tile([128, 856], f32, tag="accB", name="pb")
                        wv3 = wts[wi][:].rearrange(
                            "p (two r m) -> p two r m", two=2, r=R)
                        zv3 = zt[0][:].rearrange(
                            "p (two r n) -> p two r n", two=2, r=R)
                        for c0, cw in CH:
                            dst = (pa[:, c0:c0 + cw] if c0 < 1024
                                   else pb[:, c0 - 1024:c0 - 1024 + cw])
                            nc.tensor.matmul(
                                out=dst,
                                lhsT=wv3[:, :, r, mc * 128:mc * 128 + 128],
                                rhs=zv3[:, :, r, c0:c0 + cw],
                                start=True,
                                stop=True,
                                perf_mode=mybir.MatmulPerfMode.DoubleRow,
                            )
                        nc.scalar.copy(
                            out=st[:, mc * NPAD:mc * NPAD + 1024],
                            in_=pa[:])
                        nc.vector.tensor_copy(
                            out=st[:, mc * NPAD + 1024:(mc + 1) * NPAD],
                            in_=pb[:])
                    nc.sync.dma_start(
                        out=o[r].rearrange("(mc p) n -> p mc n", mc=2),
                        in_=st[:].rearrange("p (mc n) -> p mc n", mc=2),
                    )
    nc.finalize()   # Bacc.compile(): legalizes multi-sem waits (1/inst on TRN2)
    return nc


def _device_qkv(z):
    """z [R, N2, 256] f32 -> q, k, v [R, N2, 256] via the 8-core kernel."""
    global _compiled, LAST_RESULTS
    if _compiled is None:
        _compiled = _build_device_program()
    nc = _compiled
    zb = (z * SZ).astype(FP8)
    in_maps = []
    for d in range(NCORES):
        zs = zb[:, d * NPC:(d + 1) * NPC, :]                 # [5, 1875, 256]
        zt = np.zeros((R, HC0, NPAD), dtype=FP8)
        zt[:, :, :NPC] = zs.transpose(0, 2, 1)
        in_maps.append({"zT": zt, "wq": _W[0], "wk": _W[1], "wv": _W[2]})
    res = run_bass_kernel_spmd(
        nc, in_maps, list(range(NCORES)),
        trace=bool(os.environ.get("KERNEL_TRACE")) and _ensure_ntff_hook(),
    )
    LAST_RESULTS = res
    q = np.empty((R, N2, HC0), dtype=np.float32)
    k = np.empty((R, N2, HC0), dtype=np.float32)
    v = np.empty((R, N2, HC0), dtype=np.float32)
    for d in range(NCORES):
        rd = res.results[d]
        sl = slice(d * NPC, (d + 1) * NPC)
        q[:, sl, :] = rd["qT"][:, :, :NPC].transpose(0, 2, 1).astype(np.float32)
        k[:, sl, :] = rd["kT"][:, :, :NPC].transpose(0, 2, 1).astype(np.float32)
        v[:, sl, :] = rd["vT"][:, :, :NPC].transpose(0, 2, 1).astype(np.float32)
    inv = np.float32(1.0 / (SZ * SW))
    return q * inv, k * inv, v * inv


_W = None


def _seg_softmax_scatter(alpha, xj, seg, nseg, hc):
    """Edge softmax grouped by seg, then weighted scatter-add of xj.

    Sort-by-segment + reduceat: identical math to segment_max/segment_sum
    (empty segments yield zero rows), much faster than np.add.at.
    """
    E, H = alpha.shape
    order = np.argsort(seg, kind="stable")
    seg_s = seg[order]
    alpha_s = alpha[order]
    starts = np.flatnonzero(np.r_[True, seg_s[1:] != seg_s[:-1]])
    uniq = seg_s[starts]
    amax = np.zeros((nseg, H), dtype=np.float32)
    amax[uniq] = np.maximum.reduceat(alpha_s, starts, axis=0)
    ex_s = np.exp(alpha_s - amax[seg_s], dtype=np.float32)
    den = np.zeros((nseg, H), dtype=np.float32)
    den[uniq] = np.add.reduceat(ex_s, starts, axis=0)
    w_s = ex_s / np.maximum(den[seg_s], 1e-16)
    msg_s = (w_s[:, :, None] * xj[order].reshape(E, H, -1)).reshape(E, hc)
    z = np.zeros((nseg, hc), dtype=np.float32)
    z[uniq] = np.add.reduceat(msg_s.astype(np.float32), starts, axis=0)
    return z


def _relation_attention(z, q, k, v, Wrel, heads, outc, N):
    hc = heads * outc
    qh = q.reshape(R, N, heads, outc)
    kh = k.reshape(R, N, heads, outc)
    vh = v.reshape(R, N, heads, outc)
    psi = np.einsum("rnhc,snhc->rsnh", qh, kh).astype(np.float32)
    mask = (psi == 0) & (np.sum(psi, axis=1, keepdims=True) != 0)
    psi_m = np.where(mask, -np.inf, psi)
    pm = np.max(psi_m, axis=1, keepdims=True)
    pe = np.exp(psi_m - pm, dtype=np.float32)
    prob = pe / np.sum(pe, axis=1, keepdims=True)
    delta = np.einsum("rsnh,snhc->rnhc", prob, vh).reshape(R, N, hc)
    return np.einsum("rnd,r->nd", delta, Wrel[:, 0]).astype(np.float32)


def kernel(**inputs):
    global _W
    I = {k: np.asarray(val) for k, val in inputs.items()}
    emb = I["emb"].astype(np.float32)
    nid = I["n_id"].astype(np.int64)
    lni = I["local_node_idx"].astype(np.int64)

    # ---- group_input (only the 30000 rows that matter)
    x = emb[lni[nid[:N1]]]                                   # [30000, 128]

    # ---- layer 0: per-relation GAT over edges with tgt < 15000
    ei0 = I["edge_index0"].astype(np.int64)
    et0 = I["edge_type0"].astype(np.int64)
    keep = ei0[1] < N2
    src, tgt, rel = ei0[0][keep], ei0[1][keep], et0[keep]

    Wj0, Wi0 = I["Wj0"].astype(np.float32), I["Wi0"].astype(np.float32)
    att_j0, att_i0 = I["att_j0"].astype(np.float32), I["att_i0"].astype(np.float32)
    hj = (x @ Wj0).astype(np.float32)                        # [30000, 256]
    hi = (x[:N2] @ Wi0).astype(np.float32)                   # [15000, 256]
    H0, C0 = 4, 64
    xj = hj[src]                                             # [E, 256]
    xi = hi[tgt]
    aj = np.einsum("ehc,ehc->eh", att_j0[rel], xj.reshape(-1, H0, C0))
    ai = np.einsum("ehc,ehc->eh", att_i0[rel], xi.reshape(-1, H0, C0))
    s = (aj + ai).astype(np.float32)
    alpha = np.where(s >= 0, s, NEG_SLOPE * s).astype(np.float32)
    seg = tgt * R + rel
    z = _seg_softmax_scatter(alpha, xj, seg, N2 * R, HC0)
    z = z.reshape(N2, R, HC0).transpose(1, 0, 2)             # [5, 15000, 256]

    # ---- device: per-relation Q/K/V projections (the dominant dense block)
    _W = (np.ascontiguousarray(I["Wq0"].astype(np.float32) * SW).astype(FP8),
          np.ascontiguousarray(I["Wk0"].astype(np.float32) * SW).astype(FP8),
          np.ascontiguousarray(I["Wv0"].astype(np.float32) * SW).astype(FP8))
    try:
        q, k, v = _device_qkv(z)
    except Exception as e:  # device unavailable -> host fallback, stays correct
        sys.stderr.write(f"[kernel] device path failed ({e!r}); host fallback\n")
        W = [w.astype(np.float32) / SW for w in _W]
        q = np.einsum("rnd,rde->rne", z, W[0]).astype(np.float32)
        k = np.einsum("rnd,rde->rne", z, W[1]).astype(np.float32)
        v = np.einsum("rnd,rde->rne", z, W[2]).astype(np.float32)

    out0 = _relation_attention(z, q, k, v, I["Wrel0"].astype(np.float32), H0, C0, N2)
    x1 = out0 + x[:N2] @ I["sw0"].astype(np.float32) + I["sb0"].astype(np.float32)
    x1 = np.maximum(x1, 0.0).astype(np.float32)              # [15000, 256]

    # ---- layer 1 (small: 40-dim), host
    ei1 = I["edge_index1"].astype(np.int64)
    et1 = I["edge_type1"].astype(np.int64)
    src1, tgt1, rel1 = ei1[0], ei1[1], et1
    Wj1, Wi1 = I["Wj1"].astype(np.float32), I["Wi1"].astype(np.float32)
    hj1 = (x1 @ Wj1).astype(np.float32)                      # [15000, 40]
    hi1 = (x1[:N2] @ Wi1).astype(np.float32)
    H1, C1 = 1, 40
    xj1 = hj1[src1]
    xi1 = hi1[tgt1]
    aj1 = np.einsum("ehc,ehc->eh", I["att_j1"].astype(np.float32)[rel1],
                    xj1.reshape(-1, H1, C1))
    ai1 = np.einsum("ehc,ehc->eh", I["att_i1"].astype(np.float32)[rel1],
                    xi1.reshape(-1, H1, C1))
    s1 = (aj1 + ai1).astype(np.float32)
    alpha1 = np.where(s1 >= 0, s1, NEG_SLOPE * s1).astype(np.float32)
    seg1 = tgt1 * R + rel1
    z1 = _seg_softmax_scatter(alpha1, xj1, seg1, N2 * R, C1)
    z1 = z1.reshape(N2, R, C1).transpose(1, 0, 2)            # [5, 15000, 40]

    q1 = np.einsum("rnd,rde->rne", z1, I["Wq1"].astype(np.float32))
    k1 = np.einsum("rnd,rde->rne", z1, I["Wk1"].astype(np.float32))
    v1 = np.einsum("rnd,rde->rne", z1, I["Wv1"].astype(np.float32))
    out1 = _relation_attention(z1, q1, k1, v1, I["Wrel1"].astype(np.float32),
                               H1, C1, N2)
    x2 = out1 + x1 @ I["sw1"].astype(np.float32) + I["sb1"].astype(np.float32)

    # ---- log_softmax
    m = np.max(x2, axis=-1, keepdims=True)
    e = np.exp(x2 - m, dtype=np.float32)
    return (x2 - m - np.log(np.sum(e, axis=-1, keepdims=True))).astype(np.float32)
